# revision 1
# baseline (speedup 1.0000x reference)
"""NonLocalBlock (self-attention over 64x64 image, C=256, D=32) on 8 trn2 cores.

Sharding: data-parallel over B=4 batches x 2-way split of the attention
rows (the `n` axis of beta[n, m]) => 8 cores, each computing a [2048, 256]
slice of the output. Each core receives its batch image pre-transposed
(and fp16-cast) by the host, rolled so its own 2048 rows come first,
plus its own half in natural layout for the residual. The host also
pre-casts the 1x1-conv weights and folds gamma into Wv.

Device math (transposed attention so softmax runs along PSUM partitions):
  betaT[m, n] = q_m . k_n      (logits; no max subtraction: |L| < ~40)
  E = exp(betaT)               (ACT, bf16 out)
  oT[d, n] = sum_m v_aug[m, d] E[m, n]   with v_aug[:, 32] == 1
             => row 32 of oT is the softmax denominator
  F[n, c] = sum_d oT[d, n] Wv_aug[d, c]  with Wv_aug[:, 256] = [0..0, 1]
             => col 256 of F is the denominator, per-partition
  out[n, c] = F[n, c] / F[n, 256] + x[n, c]
"""

from contextlib import ExitStack

import ml_dtypes
import numpy as np

import concourse.bass as bass
import concourse.tile as tile
from concourse import bacc, mybir
from concourse.bass_utils import run_bass_kernel_spmd

B, H, W, C = 4, 64, 64, 256
N = H * W            # 4096 pixels per image
D = 32               # reduced channel dim
NH = N // 2          # rows owned by each core
P = 128
MT = N // P          # 32 query (m) tiles
NG = 2               # n-groups per core
GN = NH // NG        # 1024 columns per n-group
FP32 = mybir.dt.float32
BF16 = mybir.dt.bfloat16
FP16 = mybir.dt.float16
NCORES = 8

LAST_RESULTS = None  # BassKernelResults of the most recent run (for test.py)


def _body(ctx, tc, out_d, xh_d, xt_d, wf_d, wg_d, wh_d, wv_d):
    nc = tc.nc
    const = ctx.enter_context(tc.tile_pool(name="const", bufs=1))
    big = ctx.enter_context(tc.tile_pool(name="big", bufs=1))
    expp = ctx.enter_context(tc.tile_pool(name="expp", bufs=8))
    osb = ctx.enter_context(tc.tile_pool(name="osb", bufs=2))
    fin = ctx.enter_context(tc.tile_pool(name="fin", bufs=3))
    ps_beta = ctx.enter_context(tc.tile_pool(name="ps_beta", bufs=3, space="PSUM"))
    ps_o = ctx.enter_context(tc.tile_pool(name="ps_o", bufs=1, space="PSUM"))

    # ---- inputs straight from DRAM (host pre-packed) ----
    # tiny weights first (instant transfers), on the Activation HWDGE queue
    w_sb = {}
    for name, wd in (("f", wf_d), ("g", wg_d), ("h", wh_d)):
        wb = const.tile([P, 2, D], FP16, tag=f"w{name}")
        nc.scalar.dma_start(wb[:], wd.rearrange("c p d -> p c d"))
        w_sb[name] = wb
    wv_aug = const.tile([D + 1, C + 1], BF16)
    nc.scalar.dma_start(wv_aug[:], wv_d[:, :])

    xt = big.tile([P, 2, N], FP16)  # xT: [c (2 chunks of 128), m]
    # k-critical columns first; finest pieces for the very first beta deps
    pieces = [(0, 512), (512, 1024), (1024, 2048), (2048, 3072), (3072, 4096)]
    for a, b in pieces:
        for ch in range(2):
            nc.sync.dma_start(xt[:, ch, a:b], xt_d[ch, :, a:b])
    x_half = big.tile([P, NH // P, C], FP32)

    qt = big.tile([D, N], FP16)
    kt_sb = big.tile([D, NH], FP16)
    v_sb = big.tile([P, MT, D + 1], BF16)
    nc.vector.memset(v_sb[:, :, D : D + 1], 1.0)

    def proj_mg(w, dst, mg, nm, on_act=False):
        pp = ps_beta.tile([D, 512], FP32, tag="beta", name=f"p{nm}{mg}")
        for ch in range(2):
            nc.tensor.matmul(
                pp[:], w[:, ch, :], xt[:, ch, mg * 512 : (mg + 1) * 512],
                start=(ch == 0), stop=(ch == 1),
            )
        if on_act:
            nc.scalar.copy(dst[:, mg * 512 : (mg + 1) * 512], pp[:])
        else:
            nc.vector.tensor_copy(dst[:, mg * 512 : (mg + 1) * 512], pp[:])

    def v_batch(mtg, on_act=False):
        pv = ps_beta.tile([P, 4, D], FP32, tag="beta", name=f"pv{mtg}")
        for j in range(4):
            mt = mtg * 4 + j
            for ch in range(2):
                nc.tensor.matmul(
                    pv[:, j, :], xt[:, ch, mt * P : (mt + 1) * P],
                    w_sb["h"][:, ch, :],
                    start=(ch == 0), stop=(ch == 1),
                )
        cp = nc.scalar.copy if on_act else nc.vector.tensor_copy
        cp(v_sb[:, mtg * 4 : (mtg + 1) * 4, 0:D], pv[:])

    # PE p-state warmup: tiny matmuls on the weight tile, output overwritten
    warm = ps_beta.tile([P, 64], FP32, tag="beta", name="warm")
    for i in range(24):
        nc.tensor.matmul(
            warm[0:D, 0:D], w_sb["f"][:, 0, :], w_sb["f"][:, 0, 0:D],
            start=True, stop=True, skip_group_check=True,
        )
    nc.vector.tensor_copy(v_sb[0:D, 0, 0:D], warm[0:D, 0:D])  # keep it live

    # k columns for group 0 (and q for the first tiles) before the first beta
    for mg in range(2):
        proj_mg(w_sb["g"], kt_sb, mg, "k", on_act=True)
    proj_mg(w_sb["f"], qt, 0, "q", on_act=True)
    proj_mg(w_sb["f"], qt, 1, "q", on_act=True)
    v_batch(0, on_act=True)
    xh_src = xh_d.rearrange("(s p) c -> p s c", p=P)
    for piece in range(4):
        nc.gpsimd.dma_start(
            x_half[:, piece * 4 : (piece + 1) * 4, :],
            xh_src[:, piece * 4 : (piece + 1) * 4, :],
        )

    def final_sub(g, ot, out_sb, s):
        gs = g * (GN // P) + s
        if g == 1 and s % 2 == 0:
            f_ps = ps_o.tile([P, C + 1], FP32, tag="o", name=f"f{g}_{s}")
        else:
            f_ps = ps_beta.tile([P, C + 1], FP32, tag="beta", name=f"f{g}_{s}")
        nc.tensor.matmul(
            f_ps[:], ot[:, s * P : (s + 1) * P], wv_aug[:],
            start=True, stop=True,
        )
        rec = fin.tile([P, 1], FP32, tag="rec")
        nc.vector.reciprocal(rec[:], f_ps[:, C : C + 1])
        ob = out_sb[:, s, :]
        nc.vector.tensor_scalar_mul(ob, f_ps[:, 0:C], rec[:])
        if s % 2 == 0:
            nc.vector.tensor_add(ob, ob, x_half[:, gs, :])
        else:
            nc.gpsimd.tensor_add(ob, ob, x_half[:, gs, :])
        nc.sync.dma_start(out_d[gs * P : (gs + 1) * P, :], ob)

    # ---- attention main loop (group 0 streams the rest of the prologue) ----
    for g in range(NG):
        o_ps = ps_o.tile([D + 1, GN], FP32, tag="o", name=f"ops{g}")
        o_halves = [o_ps[:, 0:512], o_ps[:, 512:1024]]
        for mt in range(MT):
            if g == 0:
                mg = mt // 4 + 2  # two 512-col blocks ahead
                ph = mt % 4
                if mg < MT // 4:
                    if ph == 0:
                        proj_mg(w_sb["f"], qt, mg, "q")
                    elif ph == 1 and mg < NH // 512:
                        proj_mg(w_sb["g"], kt_sb, mg, "k")
                if ph == 2 and mt // 4 + 1 < MT // 4:
                    v_batch(mt // 4 + 1)
            pb = ps_beta.tile([P, GN], FP32, tag="beta")
            for hf in range(2):
                nc.tensor.matmul(
                    pb[:, hf * 512 : (hf + 1) * 512],
                    qt[:, mt * P : (mt + 1) * P],
                    kt_sb[:, g * GN + hf * 512 : g * GN + (hf + 1) * 512],
                    start=True, stop=True,
                )
            eb = expp.tile([P, GN], BF16, tag="exp")
            nc.scalar.activation(eb[:], pb[:], mybir.ActivationFunctionType.Exp)
            for hf in range(2):
                nc.tensor.matmul(
                    o_halves[hf],
                    v_sb[:, mt, :],
                    eb[:, hf * 512 : (hf + 1) * 512],
                    start=(mt == 0), stop=(mt == MT - 1),
                )
            if g == 1 and mt < GN // P:
                final_sub(0, prev_ot, prev_out_sb, mt)
        ot = osb.tile([D + 1, GN], BF16, tag="ot", name=f"ot{g}")
        for qtr in range(4):
            nc.vector.tensor_copy(
                ot[:, qtr * 256 : (qtr + 1) * 256],
                o_ps[:, qtr * 256 : (qtr + 1) * 256],
            )
        out_sb = fin.tile([P, GN // P, C], FP32, tag="outsb", name=f"osb{g}")
        if g == 0:
            prev_ot, prev_out_sb = ot, out_sb
        else:
            for s in range(GN // P):
                final_sub(1, ot, out_sb, s)


def build_program():
    nc = bacc.Bacc(
        "TRN2",
        target_bir_lowering=False,
        debug=False,
        enable_asserts=False,
        num_devices=NCORES,
    )
    xh_d = nc.dram_tensor("xh", [NH, C], FP32, kind="ExternalInput").ap()
    xt_d = nc.dram_tensor("xt", [2, P, N], FP16, kind="ExternalInput").ap()
    wf_d = nc.dram_tensor("Wf16", [2, P, D], FP16, kind="ExternalInput").ap()
    wg_d = nc.dram_tensor("Wg16", [2, P, D], FP16, kind="ExternalInput").ap()
    wh_d = nc.dram_tensor("Wh16", [2, P, D], FP16, kind="ExternalInput").ap()
    wv_d = nc.dram_tensor("WvAug", [D + 1, C + 1], BF16, kind="ExternalInput").ap()
    out_d = nc.dram_tensor("out", [NH, C], FP32, kind="ExternalOutput").ap()

    with tile.TileContext(nc) as tc:
        with ExitStack() as ctx:
            _body(ctx, tc, out_d, xh_d, xt_d, wf_d, wg_d, wh_d, wv_d)
    nc.compile()
    return nc


_CACHE = {}


def _get_program():
    if "nc" not in _CACHE:
        _CACHE["nc"] = build_program()
    return _CACHE["nc"]


def make_in_maps(inputs):
    x = np.ascontiguousarray(np.asarray(inputs["x"], np.float32)).reshape(B, N, C)
    gam = np.float32(np.asarray(inputs["gamma"], np.float32).reshape(()))
    w16 = {}
    for nm in ("Wf", "Wg", "Wh"):
        w = np.asarray(inputs[nm], np.float32).astype(np.float16)  # [256, 32]
        w16[nm] = np.ascontiguousarray(w.reshape(2, P, D))
    wv_aug = np.zeros((D + 1, C + 1), ml_dtypes.bfloat16)
    wv_aug[0:D, 0:C] = (gam * np.asarray(inputs["Wv"], np.float32)).astype(
        ml_dtypes.bfloat16
    )
    wv_aug[D, C] = 1.0

    in_maps = []
    for c in range(NCORES):
        b, h = divmod(c, 2)
        if h == 0:
            xb = x[b]
        else:
            xb = np.concatenate([x[b, NH:], x[b, :NH]], axis=0)
        xt = np.ascontiguousarray(xb.T.astype(np.float16).reshape(2, P, N))
        in_maps.append(
            {
                "xh": np.ascontiguousarray(xb[:NH]),
                "xt": xt,
                "Wf16": w16["Wf"],
                "Wg16": w16["Wg"],
                "Wh16": w16["Wh"],
                "WvAug": wv_aug,
            }
        )
    return in_maps


def kernel(**inputs):
    global LAST_RESULTS
    nc = _get_program()
    in_maps = make_in_maps(inputs)
    res = run_bass_kernel_spmd(nc, in_maps, core_ids=list(range(NCORES)))
    LAST_RESULTS = res
    out = np.empty((B, N, C), np.float32)
    for c in range(NCORES):
        b, h = divmod(c, 2)
        out[b, h * NH : (h + 1) * NH] = res.results[c]["out"]
    return out.reshape(B, H, W, C)



# revision 5
# speedup vs baseline: 1.1035x; 1.1035x over previous
"""NonLocalBlock (self-attention over 64x64 image, C=256, D=32) on 8 trn2 cores.

Sharding: data-parallel over B=4 batches x 2-way split of the attention
rows (the `n` axis of beta[n, m]) => 8 cores, each computing a [2048, 256]
slice of the output. Each core receives its batch image pre-transposed
(and fp16-cast) by the host, rolled so its own 2048 rows come first,
plus its own half in natural layout for the residual. The host also
pre-casts the 1x1-conv weights and folds gamma into Wv.

Device math (per core, n = its 2048 key rows, m = all 4096 queries):
  logits[m, n] = q_m . k_n               PE, fp16, [128m x 512n] matmuls
  E[m, n] = exp(logits)                  ACT exact exp + DVE exp2 bit-trick
                                         (int16(l*128*log2e + 16250.875)
                                          bitcast to bf16, trunc-fitted)
  o[n, :] = sum_m E[m, n] v_aug[m, :]    PE, E stationary (33 cols/matmul),
                                         j=mt terms issued right after each
                                         exp tile so chains finish in-group
  obar = o[:, 0:32] / o[:, 32]           DVE reciprocal + scale -> bf16
  oT = transpose(obar)                   PE (identity-moving), 32-row bands
  out[n, :] = oT.T @ (gamma Wv) + x      PE matmul + ACT/DVE PSUM->SBUF copy
                                         + Pool residual add (SBUF only)
"""

from contextlib import ExitStack

import ml_dtypes
import numpy as np

import concourse.bass as bass
import concourse.tile as tile
from concourse import bacc, mybir
from concourse.bass_utils import run_bass_kernel_spmd

B, H, W, C = 4, 64, 64, 256
N = H * W            # 4096 pixels per image
D = 32               # reduced channel dim
NH = N // 2          # key rows owned by each core
P = 128
MT = N // P          # 32 query (m) tiles
NT = NH // P         # 16 n-tiles of 128 per core
SG = 2               # supergroups of 1024 n-columns
SGW = NH // SG       # 1024
FP32 = mybir.dt.float32
BF16 = mybir.dt.bfloat16
FP16 = mybir.dt.float16
I16 = mybir.dt.int16
NCORES = 8

# exp(l) ~= bf16-bitcast(int16(l * 128*log2(e) + 16250.875)); the int16
# convert truncates, constant fitted for that (max rel err 3.3%)
EXP_S1 = float(np.float32(128 * 1.4426950408889634))
EXP_S2 = 16250.875
Aop = mybir.AluOpType

LAST_RESULTS = None  # BassKernelResults of the most recent run (for test.py)


def _exp_pattern(n_act=17, n_dve=15):
    """Weighted round-robin ACT/DVE assignment for exp tiles."""
    counts = {"A": float(n_act), "D": float(n_dve)}
    total = sum(counts.values())
    acc = dict.fromkeys(counts, 0.0)
    seq = []
    for _ in range(int(total)):
        for k in counts:
            acc[k] += counts[k] / total
        pick = max(acc, key=lambda k: acc[k])
        acc[pick] -= 1.0
        seq.append(pick)
    return seq


def _body(ctx, tc, out_d, xh_d, xt_d, wf_d, wg_d, wh_d, wv_d, id_d):
    nc = tc.nc
    const = ctx.enter_context(tc.tile_pool(name="const", bufs=1))
    big = ctx.enter_context(tc.tile_pool(name="big", bufs=1))
    obp = ctx.enter_context(tc.tile_pool(name="obp", bufs=2))
    otp = ctx.enter_context(tc.tile_pool(name="otp", bufs=2))
    rcp = ctx.enter_context(tc.tile_pool(name="rcp", bufs=4))
    fin = ctx.enter_context(tc.tile_pool(name="fin", bufs=3))
    ps = ctx.enter_context(tc.tile_pool(name="ps", bufs=3, space="PSUM"))
    sm = ctx.enter_context(tc.tile_pool(name="sm", bufs=2, space="PSUM"))

    # ---- tiny weights first (instant transfers) on the ACT HWDGE queue ----
    w_sb = {}
    for name, wd in (("f", wf_d), ("g", wg_d), ("h", wh_d)):
        wb = const.tile([P, 2, D], FP16, tag=f"w{name}")
        nc.scalar.dma_start(wb[:], wd.rearrange("c p d -> p c d"))
        w_sb[name] = wb
    wv = const.tile([D, C], BF16)
    nc.scalar.dma_start(wv[:], wv_d)
    ident = const.tile([P, P], BF16)
    nc.scalar.dma_start(ident[:], id_d)

    xt = big.tile([P, 2, N], FP16)  # xT: [c (2 chunks of 128), m]
    pieces = [(0, 512), (512, 1024), (1024, 2048), (2048, 3072), (3072, 4096)]
    for a, b in pieces:
        for ch in range(2):
            nc.sync.dma_start(xt[:, ch, a:b], xt_d[ch, :, a:b])
    x_half = big.tile([P, NT, C], FP32)
    qt = big.tile([D, N], FP16)            # q: [d, m]
    kt = big.tile([D, NH], FP16)           # k: [d, n] (own half only)
    v_sb = big.tile([P, MT, D + 1], BF16)  # v: [m, d | 1]
    e_sb = big.tile([P, MT, NH], BF16)     # E: [m-part, mt, n]
    nc.vector.memset(v_sb[:, :, D:D + 1], 1.0)

    def proj(w, dst, mg, nm, on_act=False):
        pp = ps.tile([D, 512], FP32, tag="ps", name=f"p{nm}{mg}")
        for ch in range(2):
            nc.tensor.matmul(
                pp[:], w[:, ch, :], xt[:, ch, mg * 512:(mg + 1) * 512],
                start=(ch == 0), stop=(ch == 1),
            )
        if on_act:
            nc.scalar.copy(dst[:, mg * 512:(mg + 1) * 512], pp[:])
        else:
            nc.vector.tensor_copy(dst[:, mg * 512:(mg + 1) * 512], pp[:])

    def v_batch(mtg, on_act=False):
        pv = ps.tile([P, 4, D], FP32, tag="ps", name=f"pv{mtg}")
        for j in range(4):
            mt = mtg * 4 + j
            for ch in range(2):
                nc.tensor.matmul(
                    pv[:, j, :], xt[:, ch, mt * P:(mt + 1) * P],
                    w_sb["h"][:, ch, :],
                    start=(ch == 0), stop=(ch == 1),
                )
        if on_act:
            nc.scalar.copy(v_sb[:, mtg * 4:(mtg + 1) * 4, 0:D], pv[:])
        else:
            nc.vector.tensor_copy(v_sb[:, mtg * 4:(mtg + 1) * 4, 0:D], pv[:])

    # PE p-state warmup: tiny matmuls on the weight tile, output overwritten
    warm = ps.tile([P, 64], FP32, tag="ps", name="warm")
    for _ in range(24):
        nc.tensor.matmul(
            warm[0:D, 0:D], w_sb["f"][:, 0, :], w_sb["f"][:, 0, 0:D],
            start=True, stop=True, skip_group_check=True,
        )
    nc.vector.tensor_copy(v_sb[0:D, 0, 0:D], warm[0:D, 0:D])  # keep it live

    # prologue projections (ACT is otherwise idle this early)
    proj(w_sb["f"], qt, 0, "q", on_act=True)
    proj(w_sb["g"], kt, 0, "k", on_act=True)
    proj(w_sb["f"], qt, 1, "q", on_act=True)
    proj(w_sb["g"], kt, 1, "k", on_act=False)
    v_batch(0, on_act=True)
    xh_src = xh_d.rearrange("(s p) c -> p s c", p=P)
    for piece in range(4):
        nc.gpsimd.dma_start(
            x_half[:, piece * 4:(piece + 1) * 4, :],
            xh_src[:, piece * 4:(piece + 1) * 4, :],
        )

    pat = _exp_pattern()

    def o_mms(sg, j, oB):
        # one accumulation term (query tile j) for all 8 chains of the
        # supergroup; the chains share one PSUM zero-region, so only the
        # very first matmul starts it and the very last stops it (bytes
        # zero lazily on first touch)
        for t in range(8):
            nt = sg * 8 + t
            nc.tensor.matmul(
                oB[:, t, :], e_sb[:, j, nt * P:(nt + 1) * P], v_sb[:, j, :],
                start=(j == 0 and t == 0), stop=(j == MT - 1 and t == 7),
            )

    def finals_half(sg, h2, obar4):
        # transpose 4 bands via identity-moving matmuls, then 4 final tiles
        oTps = ps.tile([D, 4, P], FP32, tag="ps", name=f"otp{sg}_{h2}")
        for bd in range(4):
            nc.tensor.matmul(oTps[:, bd, :], obar4[:, bd, :], ident[:],
                             start=True, stop=True)
        oT = otp.tile([D, 4, P], BF16, tag="ot", name=f"ot{sg}_{h2}")
        nc.vector.tensor_copy(oT[:], oTps[:])
        for bd in range(4):
            nt = sg * 8 + h2 * 4 + bd
            fps = sm.tile([P, C], FP32, tag="sm", name=f"F{nt}")
            nc.tensor.matmul(fps[:], oT[:, bd, :], wv[:], start=True, stop=True)
            osb = fin.tile([P, C], FP32, tag="osb", name=f"osb{nt}")
            if bd % 2 == 0:
                nc.scalar.copy(osb[:], fps[:])
            else:
                nc.vector.tensor_copy(osb[:], fps[:])
            nc.gpsimd.tensor_add(osb[:], osb[:], x_half[:, nt, :])
            nc.sync.dma_start(out_d[nt * P:(nt + 1) * P, :], osb[:])

    # ---- main loop ----
    for sg in range(SG):
        oB = sm.tile([P, 8, D + 1], FP32, tag="sm", name=f"oB{sg}")
        for mt in range(MT):
            if sg == 0:
                ph, mg = mt % 4, mt // 4
                if ph == 0 and mg + 2 < 8:
                    proj(w_sb["f"], qt, mg + 2, "q")
                elif ph == 1 and mg + 2 < 4:
                    proj(w_sb["g"], kt, mg + 2, "k")
                elif ph == 2 and mg + 1 < 8:
                    v_batch(mg + 1)
            pb = ps.tile([P, SGW], FP32, tag="ps", name=f"pb{sg}_{mt}")
            for hf in range(2):
                nc.tensor.matmul(
                    pb[:, hf * 512:(hf + 1) * 512],
                    qt[:, mt * P:(mt + 1) * P],
                    kt[:, sg * SGW + hf * 512:sg * SGW + (hf + 1) * 512],
                    start=True, stop=True,
                )
            e_dst = e_sb[:, mt, sg * SGW:(sg + 1) * SGW]
            if pat[mt % len(pat)] == "A":
                nc.scalar.activation(e_dst, pb[:],
                                     mybir.ActivationFunctionType.Exp)
            else:
                nc.vector.tensor_scalar(e_dst.bitcast(I16), pb[:],
                                        EXP_S1, EXP_S2, Aop.mult, Aop.add)
            if mt >= 1:
                o_mms(sg, mt - 1, oB)
        o_mms(sg, MT - 1, oB)

        # normalize the 8 chains -> two obar4 batches -> finals
        for h2 in range(2):
            obar4 = obp.tile([P, 4, D], BF16, tag="ob", name=f"ob{sg}_{h2}")
            for bd in range(4):
                t = h2 * 4 + bd
                nt = sg * 8 + t
                rec = rcp.tile([P, 1], FP32, tag="rec", name=f"rec{nt}")
                nc.vector.reciprocal(rec[:], oB[:, t, D:D + 1])
                nc.vector.tensor_scalar(
                    obar4[:, bd, :], oB[:, t, 0:D], rec[:], None, Aop.mult,
                )
            finals_half(sg, h2, obar4)


def build_program():
    nc = bacc.Bacc(
        "TRN2",
        target_bir_lowering=False,
        debug=False,
        enable_asserts=False,
        num_devices=NCORES,
    )
    xh_d = nc.dram_tensor("xh", [NH, C], FP32, kind="ExternalInput").ap()
    xt_d = nc.dram_tensor("xt", [2, P, N], FP16, kind="ExternalInput").ap()
    wf_d = nc.dram_tensor("Wf16", [2, P, D], FP16, kind="ExternalInput").ap()
    wg_d = nc.dram_tensor("Wg16", [2, P, D], FP16, kind="ExternalInput").ap()
    wh_d = nc.dram_tensor("Wh16", [2, P, D], FP16, kind="ExternalInput").ap()
    wv_d = nc.dram_tensor("WvG", [D, C], BF16, kind="ExternalInput").ap()
    id_d = nc.dram_tensor("Ident", [P, P], BF16, kind="ExternalInput").ap()
    out_d = nc.dram_tensor("out", [NH, C], FP32, kind="ExternalOutput").ap()

    with tile.TileContext(nc) as tc:
        with ExitStack() as ctx:
            _body(ctx, tc, out_d, xh_d, xt_d, wf_d, wg_d, wh_d, wv_d, id_d)
    nc.compile()
    return nc


_CACHE = {}


def _get_program():
    if "nc" not in _CACHE:
        _CACHE["nc"] = build_program()
    return _CACHE["nc"]


def make_in_maps(inputs):
    x = np.ascontiguousarray(np.asarray(inputs["x"], np.float32)).reshape(B, N, C)
    gam = np.float32(np.asarray(inputs["gamma"], np.float32).reshape(()))
    w16 = {}
    for nm in ("Wf", "Wg", "Wh"):
        w = np.asarray(inputs[nm], np.float32).astype(np.float16)  # [256, 32]
        w16[nm] = np.ascontiguousarray(w.reshape(2, P, D))
    wv = np.ascontiguousarray(
        (gam * np.asarray(inputs["Wv"], np.float32)).astype(ml_dtypes.bfloat16)
    )
    ident = np.ascontiguousarray(np.eye(P, dtype=ml_dtypes.bfloat16))

    in_maps = []
    for c in range(NCORES):
        b, h = divmod(c, 2)
        if h == 0:
            xb = x[b]
        else:
            xb = np.concatenate([x[b, NH:], x[b, :NH]], axis=0)
        xt = np.ascontiguousarray(xb.T.astype(np.float16).reshape(2, P, N))
        in_maps.append(
            {
                "xh": np.ascontiguousarray(xb[:NH]),
                "xt": xt,
                "Wf16": w16["Wf"],
                "Wg16": w16["Wg"],
                "Wh16": w16["Wh"],
                "WvG": wv,
                "Ident": ident,
            }
        )
    return in_maps


def kernel(**inputs):
    global LAST_RESULTS
    nc = _get_program()
    in_maps = make_in_maps(inputs)
    res = run_bass_kernel_spmd(nc, in_maps, core_ids=list(range(NCORES)))
    LAST_RESULTS = res
    out = np.empty((B, N, C), np.float32)
    for c in range(NCORES):
        b, h = divmod(c, 2)
        out[b, h * NH:(h + 1) * NH] = res.results[c]["out"]
    return out.reshape(B, H, W, C)


# revision 6
# speedup vs baseline: 1.1937x; 1.0817x over previous
"""NonLocalBlock (self-attention over 64x64 image, C=256, D=32) on 8 trn2 cores.

Sharding: data-parallel over B=4 batches x 2-way split of the attention
rows (the `n` axis of beta[n, m]) => 8 cores, each computing a [2048, 256]
slice of the output. Each core receives its batch image pre-transposed
(and fp16-cast) by the host, rolled so its own 2048 rows come first,
plus its own half in natural layout for the residual. The host also
pre-casts the 1x1-conv weights and folds gamma into Wv.

Device math (per core, n = its 2048 key rows, m = all 4096 queries):
  logits[m, n] = q_m . k_n               PE, fp16, [128m x 512n] matmuls
  E[m, n] = exp(logits)                  ACT exact exp + DVE exp2 bit-trick
                                         (int16(l*128*log2e + 16250.875)
                                          bitcast to bf16, trunc-fitted)
  o[n, :] = sum_m E[m, n] v_aug[m, :]    PE, E stationary (33 cols/matmul),
                                         j=mt terms issued right after each
                                         exp tile so chains finish in-group
  obar = o[:, 0:32] / o[:, 32]           DVE reciprocal + scale -> bf16
  oT = transpose(obar)                   PE (identity-moving), 32-row bands
  out[n, :] = oT.T @ (gamma Wv) + x      PE matmul + ACT/DVE PSUM->SBUF copy
                                         + Pool residual add (SBUF only)
"""

from contextlib import ExitStack

import ml_dtypes
import numpy as np

import concourse.bass as bass
import concourse.tile as tile
from concourse import bacc, mybir
from concourse.bass_utils import run_bass_kernel_spmd

B, H, W, C = 4, 64, 64, 256
N = H * W            # 4096 pixels per image
D = 32               # reduced channel dim
NH = N // 2          # key rows owned by each core
P = 128
MT = N // P          # 32 query (m) tiles
NT = NH // P         # 16 n-tiles of 128 per core
SG = 2               # supergroups of 1024 n-columns
SGW = NH // SG       # 1024
FP32 = mybir.dt.float32
BF16 = mybir.dt.bfloat16
FP16 = mybir.dt.float16
I16 = mybir.dt.int16
NCORES = 8

# exp(l) ~= bf16-bitcast(int16(l * 128*log2(e) + 16250.875)); the int16
# convert truncates, constant fitted for that (max rel err 3.3%)
EXP_S1 = float(np.float32(128 * 1.4426950408889634))
EXP_S2 = 16250.875
Aop = mybir.AluOpType

LAST_RESULTS = None  # BassKernelResults of the most recent run (for test.py)


def _exp_pattern(n_act=19, n_dve=13):
    """Weighted round-robin ACT/DVE assignment for exp tiles."""
    counts = {"A": float(n_act), "D": float(n_dve)}
    total = sum(counts.values())
    acc = dict.fromkeys(counts, 0.0)
    seq = []
    for _ in range(int(total)):
        for k in counts:
            acc[k] += counts[k] / total
        pick = max(acc, key=lambda k: acc[k])
        acc[pick] -= 1.0
        seq.append(pick)
    return seq


def _body(ctx, tc, out_d, xh_d, xt_d, wf_d, wg_d, wh_d, wv_d, id_d):
    nc = tc.nc
    const = ctx.enter_context(tc.tile_pool(name="const", bufs=1))
    big = ctx.enter_context(tc.tile_pool(name="big", bufs=1))
    obp = ctx.enter_context(tc.tile_pool(name="obp", bufs=2))
    otp = ctx.enter_context(tc.tile_pool(name="otp", bufs=2))
    rcp = ctx.enter_context(tc.tile_pool(name="rcp", bufs=4))
    fin = ctx.enter_context(tc.tile_pool(name="fin", bufs=3))
    ps = ctx.enter_context(tc.tile_pool(name="ps", bufs=3, space="PSUM"))
    sm = ctx.enter_context(tc.tile_pool(name="sm", bufs=2, space="PSUM"))

    # ---- tiny weights first (instant transfers) on the ACT HWDGE queue ----
    w_sb = {}
    for name, wd in (("f", wf_d), ("g", wg_d), ("h", wh_d)):
        wb = const.tile([P, 2, D], FP16, tag=f"w{name}")
        nc.scalar.dma_start(wb[:], wd.rearrange("c p d -> p c d"))
        w_sb[name] = wb
    wv = const.tile([D, C], BF16)
    nc.scalar.dma_start(wv[:], wv_d)
    ident = const.tile([P, P], BF16)
    nc.scalar.dma_start(ident[:], id_d)

    xt = big.tile([P, 2, N], FP16)  # xT: [c (2 chunks of 128), m]
    pieces = [(0, 512), (512, 1024), (1024, 2048), (2048, 3072), (3072, 4096)]
    for a, b in pieces:
        for ch in range(2):
            nc.sync.dma_start(xt[:, ch, a:b], xt_d[ch, :, a:b])
    x_half = big.tile([P, NT, C], FP32)
    qt = big.tile([D, N], FP16)            # q: [d, m]
    kt = big.tile([D, NH], FP16)           # k: [d, n] (own half only)
    v_sb = big.tile([P, MT, D + 1], BF16)  # v: [m, d | 1]
    e_sb = big.tile([P, MT, NH], BF16)     # E: [m-part, mt, n]
    nc.vector.memset(v_sb[:, :, D:D + 1], 1.0)

    def proj(w, dst, mg, nm, on_act=False):
        pp = ps.tile([D, 512], FP32, tag="ps", name=f"p{nm}{mg}")
        for ch in range(2):
            nc.tensor.matmul(
                pp[:], w[:, ch, :], xt[:, ch, mg * 512:(mg + 1) * 512],
                start=(ch == 0), stop=(ch == 1),
            )
        if on_act:
            nc.scalar.copy(dst[:, mg * 512:(mg + 1) * 512], pp[:])
        else:
            nc.vector.tensor_copy(dst[:, mg * 512:(mg + 1) * 512], pp[:])

    def v_batch(mtg, on_act=False):
        pv = ps.tile([P, 4, D], FP32, tag="ps", name=f"pv{mtg}")
        for j in range(4):
            mt = mtg * 4 + j
            for ch in range(2):
                nc.tensor.matmul(
                    pv[:, j, :], xt[:, ch, mt * P:(mt + 1) * P],
                    w_sb["h"][:, ch, :],
                    start=(ch == 0), stop=(ch == 1),
                )
        if on_act:
            nc.scalar.copy(v_sb[:, mtg * 4:(mtg + 1) * 4, 0:D], pv[:])
        else:
            nc.vector.tensor_copy(v_sb[:, mtg * 4:(mtg + 1) * 4, 0:D], pv[:])

    # PE p-state warmup: tiny matmuls on the weight tile, output overwritten
    warm = ps.tile([P, 64], FP32, tag="ps", name="warm")
    for _ in range(24):
        nc.tensor.matmul(
            warm[0:D, 0:D], w_sb["f"][:, 0, :], w_sb["f"][:, 0, 0:D],
            start=True, stop=True, skip_group_check=True,
        )
    nc.vector.tensor_copy(v_sb[0:D, 0, 0:D], warm[0:D, 0:D])  # keep it live

    # prologue projections (ACT is otherwise idle this early)
    proj(w_sb["f"], qt, 0, "q", on_act=True)
    proj(w_sb["g"], kt, 0, "k", on_act=True)
    proj(w_sb["f"], qt, 1, "q", on_act=True)
    proj(w_sb["g"], kt, 1, "k", on_act=False)
    v_batch(0, on_act=True)
    xh_src = xh_d.rearrange("(s p) c -> p s c", p=P)
    for piece in range(4):
        nc.gpsimd.dma_start(
            x_half[:, piece * 4:(piece + 1) * 4, :],
            xh_src[:, piece * 4:(piece + 1) * 4, :],
        )

    pat = _exp_pattern()

    def o_mms(sg, j, oB):
        # one accumulation term (query tile j) for all 8 chains of the
        # supergroup; the chains share one PSUM zero-region, so only the
        # very first matmul starts it and the very last stops it (bytes
        # zero lazily on first touch)
        for t in range(8):
            nt = sg * 8 + t
            nc.tensor.matmul(
                oB[:, t, :], e_sb[:, j, nt * P:(nt + 1) * P], v_sb[:, j, :],
                start=(j == 0 and t == 0), stop=(j == MT - 1 and t == 7),
            )

    def finals_half(sg, h2, obar4):
        # transpose 4 bands via identity-moving matmuls, then 4 final tiles
        oTps = ps.tile([D, 4, P], FP32, tag="ps", name=f"otp{sg}_{h2}")
        for bd in range(4):
            nc.tensor.matmul(oTps[:, bd, :], obar4[:, bd, :], ident[:],
                             start=True, stop=True)
        oT = otp.tile([D, 4, P], BF16, tag="ot", name=f"ot{sg}_{h2}")
        nc.scalar.copy(oT[:], oTps[:])
        for bd in range(4):
            nt = sg * 8 + h2 * 4 + bd
            fps = sm.tile([P, C], FP32, tag="sm", name=f"F{nt}")
            nc.tensor.matmul(fps[:], oT[:, bd, :], wv[:], start=True, stop=True)
            osb = fin.tile([P, C], FP32, tag="osb", name=f"osb{nt}")
            if bd % 2 == 0:
                nc.scalar.copy(osb[:], fps[:])
            else:
                nc.vector.tensor_copy(osb[:], fps[:])
            nc.gpsimd.tensor_add(osb[:], osb[:], x_half[:, nt, :])
            nc.sync.dma_start(out_d[nt * P:(nt + 1) * P, :], osb[:])

    def obars_half(sg, h2, oB):
        obar4 = obp.tile([P, 4, D], BF16, tag="ob", name=f"ob{sg}_{h2}")
        for bd in range(4):
            t = h2 * 4 + bd
            nt = sg * 8 + t
            rec = rcp.tile([P, 1], FP32, tag="rec", name=f"rec{nt}")
            nc.vector.reciprocal(rec[:], oB[:, t, D:D + 1])
            nc.vector.tensor_scalar(
                obar4[:, bd, :], oB[:, t, 0:D], rec[:], None, Aop.mult,
            )
        return obar4

    # ---- main loop: flat over 64 beta/exp tiles, o-chains lag 3 steps ----
    LAG = 3
    oBs = [None, None]
    obar4s = {}

    def o_step(s):
        sgp, j = divmod(s, MT)
        if j == 0:
            oBs[sgp] = sm.tile([P, 8, D + 1], FP32, tag="sm", name=f"oB{sgp}")
        o_mms(sgp, j, oBs[sgp])
        if j == MT - 1:
            obar4s[(sgp, 0)] = obars_half(sgp, 0, oBs[sgp])
            obar4s[(sgp, 1)] = obars_half(sgp, 1, oBs[sgp])

    for s in range(MT * SG):
        sg, mt = divmod(s, MT)
        if sg == 0:
            ph, mg = mt % 4, mt // 4
            if ph == 0 and mg + 2 < 8:
                proj(w_sb["f"], qt, mg + 2, "q")
            elif ph == 1 and mg + 2 < 4:
                proj(w_sb["g"], kt, mg + 2, "k")
            elif ph == 2 and mg + 1 < 8:
                v_batch(mg + 1)
        pb = ps.tile([P, SGW], FP32, tag="ps", name=f"pb{sg}_{mt}")
        for hf in range(2):
            nc.tensor.matmul(
                pb[:, hf * 512:(hf + 1) * 512],
                qt[:, mt * P:(mt + 1) * P],
                kt[:, sg * SGW + hf * 512:sg * SGW + (hf + 1) * 512],
                start=True, stop=True,
            )
        e_dst = e_sb[:, mt, sg * SGW:(sg + 1) * SGW]
        if pat[s % len(pat)] == "A":
            nc.scalar.activation(e_dst, pb[:],
                                 mybir.ActivationFunctionType.Exp)
        else:
            nc.vector.tensor_scalar(e_dst.bitcast(I16), pb[:],
                                    EXP_S1, EXP_S2, Aop.mult, Aop.add)
        if s >= LAG:
            o_step(s - LAG)
        # finals of supergroup 0 interleave into supergroup 1's steps
        if s == MT + 8:
            finals_half(0, 0, obar4s[(0, 0)])
        elif s == MT + 12:
            finals_half(0, 1, obar4s[(0, 1)])
    for s in range(MT * SG - LAG, MT * SG):
        o_step(s)
    finals_half(1, 0, obar4s[(1, 0)])
    finals_half(1, 1, obar4s[(1, 1)])


def build_program():
    nc = bacc.Bacc(
        "TRN2",
        target_bir_lowering=False,
        debug=False,
        enable_asserts=False,
        num_devices=NCORES,
    )
    xh_d = nc.dram_tensor("xh", [NH, C], FP32, kind="ExternalInput").ap()
    xt_d = nc.dram_tensor("xt", [2, P, N], FP16, kind="ExternalInput").ap()
    wf_d = nc.dram_tensor("Wf16", [2, P, D], FP16, kind="ExternalInput").ap()
    wg_d = nc.dram_tensor("Wg16", [2, P, D], FP16, kind="ExternalInput").ap()
    wh_d = nc.dram_tensor("Wh16", [2, P, D], FP16, kind="ExternalInput").ap()
    wv_d = nc.dram_tensor("WvG", [D, C], BF16, kind="ExternalInput").ap()
    id_d = nc.dram_tensor("Ident", [P, P], BF16, kind="ExternalInput").ap()
    out_d = nc.dram_tensor("out", [NH, C], FP32, kind="ExternalOutput").ap()

    with tile.TileContext(nc) as tc:
        with ExitStack() as ctx:
            _body(ctx, tc, out_d, xh_d, xt_d, wf_d, wg_d, wh_d, wv_d, id_d)
    nc.compile()
    return nc


_CACHE = {}


def _get_program():
    if "nc" not in _CACHE:
        _CACHE["nc"] = build_program()
    return _CACHE["nc"]


def make_in_maps(inputs):
    x = np.ascontiguousarray(np.asarray(inputs["x"], np.float32)).reshape(B, N, C)
    gam = np.float32(np.asarray(inputs["gamma"], np.float32).reshape(()))
    w16 = {}
    for nm in ("Wf", "Wg", "Wh"):
        w = np.asarray(inputs[nm], np.float32).astype(np.float16)  # [256, 32]
        w16[nm] = np.ascontiguousarray(w.reshape(2, P, D))
    wv = np.ascontiguousarray(
        (gam * np.asarray(inputs["Wv"], np.float32)).astype(ml_dtypes.bfloat16)
    )
    ident = np.ascontiguousarray(np.eye(P, dtype=ml_dtypes.bfloat16))

    in_maps = []
    for c in range(NCORES):
        b, h = divmod(c, 2)
        if h == 0:
            xb = x[b]
        else:
            xb = np.concatenate([x[b, NH:], x[b, :NH]], axis=0)
        xt = np.ascontiguousarray(xb.T.astype(np.float16).reshape(2, P, N))
        in_maps.append(
            {
                "xh": np.ascontiguousarray(xb[:NH]),
                "xt": xt,
                "Wf16": w16["Wf"],
                "Wg16": w16["Wg"],
                "Wh16": w16["Wh"],
                "WvG": wv,
                "Ident": ident,
            }
        )
    return in_maps


def kernel(**inputs):
    global LAST_RESULTS
    nc = _get_program()
    in_maps = make_in_maps(inputs)
    res = run_bass_kernel_spmd(nc, in_maps, core_ids=list(range(NCORES)))
    LAST_RESULTS = res
    out = np.empty((B, N, C), np.float32)
    for c in range(NCORES):
        b, h = divmod(c, 2)
        out[b, h * NH:(h + 1) * NH] = res.results[c]["out"]
    return out.reshape(B, H, W, C)


# revision 24
# speedup vs baseline: 1.3633x; 1.1421x over previous
"""NonLocalBlock (self-attention over 64x64 image, C=256, D=32) on 8 trn2 cores.

Sharding: data-parallel over B=4 batches x 2-way split of the attention
rows (the `n` axis of beta[n, m]) => 8 cores, each computing a [2048, 256]
slice of the output. Each core receives its batch image pre-transposed
(and fp16-cast) by the host, rolled so its own 2048 rows come first,
plus its own half in natural layout for the residual. The host also
pre-casts the 1x1-conv weights and folds gamma into Wv.

Device math (per core, n = its 2048 key rows, m = all 4096 queries):
  logits[m, n] = q_m . k_n               PE, fp16, [128m x 512n] matmuls
  E[m, n] = exp(logits)                  ACT exact exp + DVE exp2 bit-trick
                                         (int16(l*128*log2e + 16250.875)
                                          bitcast to bf16, trunc-fitted)
  o[n, :] = sum_m E[m, n] v_aug[m, :]    PE, E stationary (33 cols/matmul),
                                         terms issued a few steps behind
                                         each exp tile; 8 chains share one
                                         PSUM zero-region (single start/stop)
  obar = o[:, 0:32] / o[:, 32]           DVE reciprocal + ACT/DVE scale, bf16
  oT = transpose(obar)                   PE (identity-moving), 32-row bands
  out[n, :] = oT.T @ (gamma Wv) + x      PE matmul + {ACT copy + Pool add |
                                         DVE fused add}, DMA out

Engine layout: ACT exp tiles are [128,1024] double-buffered in PSUM
(2x2 banks), DVE trick tiles are [128,512] double-buffered (2x1 banks),
o-chain accumulator 1 bank, final tiles 1 bank.
"""

from contextlib import ExitStack

import ml_dtypes
import numpy as np

import concourse.bass as bass
import concourse.tile as tile
from concourse import bacc, mybir
from concourse.bass_utils import run_bass_kernel_spmd

B, H, W, C = 4, 64, 64, 256
N = H * W            # 4096 pixels per image
D = 32               # reduced channel dim
NH = N // 2          # key rows owned by each core
P = 128
MT = N // P          # 32 query (m) tiles
NT = NH // P         # 16 n-tiles of 128 per core
SG = 2               # supergroups of 1024 n-columns
SGW = NH // SG       # 1024
FP32 = mybir.dt.float32
BF16 = mybir.dt.bfloat16
FP16 = mybir.dt.float16
I16 = mybir.dt.int16
NCORES = 8

# exp(l) ~= bf16-bitcast(int16(l * 128*log2(e) + 16250.875)); the int16
# convert truncates, constant fitted for that (max rel err 3.3%)
EXP_S1 = float(np.float32(128 * 1.4426950408889634))
EXP_S2 = 16250.875
Aop = mybir.AluOpType

LAST_RESULTS = None  # BassKernelResults of the most recent run (for test.py)

LAG = 3  # steps between an exp tile and its o-chain consumption


def _exp_pattern(n_act=19, n_dve=13):
    """Weighted round-robin ACT/DVE assignment for exp tiles (per 32)."""
    counts = {"A": float(n_act), "D": float(n_dve)}
    total = sum(counts.values())
    acc = dict.fromkeys(counts, 0.0)
    seq = []
    for _ in range(int(total)):
        for k in counts:
            acc[k] += counts[k] / total
        pick = max(acc, key=lambda k: acc[k])
        acc[pick] -= 1.0
        seq.append(pick)
    return seq


def _body(ctx, tc, out_d, xh_d, xt_d, wf_d, wg_d, wh_d, wv_d, id_d):
    nc = tc.nc
    const = ctx.enter_context(tc.tile_pool(name="const", bufs=1))
    big = ctx.enter_context(tc.tile_pool(name="big", bufs=1))
    ep = ctx.enter_context(tc.tile_pool(name="ep", bufs=10))
    obp = ctx.enter_context(tc.tile_pool(name="obp", bufs=4))
    otp = ctx.enter_context(tc.tile_pool(name="otp", bufs=4))
    spp = ctx.enter_context(tc.tile_pool(name="spp", bufs=2))
    ocp = ctx.enter_context(tc.tile_pool(name="ocp", bufs=2))
    rcp = ctx.enter_context(tc.tile_pool(name="rcp", bufs=4))
    fin = ctx.enter_context(tc.tile_pool(name="fin", bufs=8))
    psA = ctx.enter_context(tc.tile_pool(name="psA", bufs=2, space="PSUM"))
    psD = ctx.enter_context(tc.tile_pool(name="psD", bufs=3, space="PSUM"))
    psO = ctx.enter_context(tc.tile_pool(name="psO", bufs=1, space="PSUM"))

    # ---- tiny weights first (instant transfers) on the ACT HWDGE queue ----
    w_sb = {}
    for name, wd in (("f", wf_d), ("g", wg_d), ("h", wh_d)):
        wb = const.tile([P, 2, D], FP16, tag=f"w{name}")
        nc.scalar.dma_start(wb[:], wd.rearrange("c p d -> p c d"))
        w_sb[name] = wb
    wvr = const.tile([P, C], BF16)
    nc.scalar.dma_start(wvr[:], wv_d)
    ident = const.tile([P, P], BF16)
    nc.scalar.dma_start(ident[:], id_d)

    xt = big.tile([P, 2, N], FP16)  # xT: [c (2 chunks of 128), m]
    pieces = [(0, 512), (512, 1024), (1024, 2048), (2048, 3072), (3072, 4096)]
    for a, b in pieces:
        for ch in range(2):
            nc.sync.dma_start(xt[:, ch, a:b], xt_d[ch, :, a:b])
    x_half = big.tile([P, NT, C], FP32)
    qt = big.tile([D, N], FP16)            # q: [d, m]
    kt = big.tile([D, NH], FP16)           # k: [d, n] (own half only)
    v_sb = big.tile([P, MT, D + 1], BF16)  # v: [m, d | 1]
    nc.vector.memset(v_sb[:, :, D:D + 1], 1.0)

    def proj_mm(w, mg, nm):
        pp = psD.tile([D, 512], FP32, tag="pd", name=f"p{nm}{mg}")
        for ch in range(2):
            nc.tensor.matmul(
                pp[:], w[:, ch, :], xt[:, ch, mg * 512:(mg + 1) * 512],
                start=(ch == 0), stop=(ch == 1),
            )
        return pp

    def proj_copy(pp, dst, mg, on_act=False):
        if on_act:
            nc.scalar.copy(dst[:, mg * 512:(mg + 1) * 512], pp[:])
        else:
            nc.vector.tensor_copy(dst[:, mg * 512:(mg + 1) * 512], pp[:])

    def proj(w, dst, mg, nm, on_act=False):
        proj_copy(proj_mm(w, mg, nm), dst, mg, on_act)

    def v_mm(mtg):
        pv = psA.tile([P, 4, D], FP32, tag="pa", name=f"pv{mtg}")
        for j in range(4):
            mt = mtg * 4 + j
            for ch in range(2):
                nc.tensor.matmul(
                    pv[:, j, :], xt[:, ch, mt * P:(mt + 1) * P],
                    w_sb["h"][:, ch, :],
                    start=(ch == 0), stop=(ch == 1),
                )
        return pv

    def v_copy(pv, mtg, on_act=False):
        if on_act:
            nc.scalar.copy(v_sb[:, mtg * 4:(mtg + 1) * 4, 0:D], pv[:])
        else:
            nc.vector.tensor_copy(v_sb[:, mtg * 4:(mtg + 1) * 4, 0:D], pv[:])

    def v_batch(mtg, on_act=False):
        v_copy(v_mm(mtg), mtg, on_act)

    # PE p-state warmup: tiny matmuls on a memset tile (no DMA dependency)
    wsrc = big.tile([P, D], BF16, tag="wsrc")
    nc.vector.memset(wsrc[:], 0.25)
    warm = psA.tile([P, 64], FP32, tag="pa", name="warm")
    for _ in range(32):
        nc.tensor.matmul(
            warm[0:D, 0:D], wsrc[:], wsrc[:, 0:D],
            start=True, stop=True, skip_group_check=True,
        )
    nc.vector.tensor_copy(v_sb[0:D, 0, 0:D], warm[0:D, 0:D])  # keep it live

    # prologue projections (ACT is otherwise idle this early)
    pq0 = proj_mm(w_sb["f"], 0, "q")
    pk0 = proj_mm(w_sb["g"], 0, "k")
    proj_copy(pq0, qt, 0, on_act=True)
    proj_copy(pk0, kt, 0, on_act=False)
    pq1 = proj_mm(w_sb["f"], 1, "q")
    pk1 = proj_mm(w_sb["g"], 1, "k")
    proj_copy(pq1, qt, 1, on_act=True)
    proj_copy(pk1, kt, 1, on_act=False)
    v_batch(0, on_act=True)
    xh_src = xh_d.rearrange("(s p) c -> p s c", p=P)

    pat = _exp_pattern()

    def o_mms(sg, j, oB, ets):
        # one accumulation term (query tile j) for all 8 chains of the
        # supergroup; the chains share one PSUM zero-region, so only the
        # very first matmul starts it and the very last stops it (bytes
        # zero lazily on first touch)
        for t in range(8):
            if len(ets) == 1:
                esrc = ets[0][:, t * P:(t + 1) * P]
            else:
                esrc = ets[t // 4][:, (t % 4) * P:(t % 4 + 1) * P]
            nc.tensor.matmul(
                oB[:, t, :], esrc, v_sb[:, j, :],
                start=(j == 0 and t == 0), stop=(j == MT - 1 and t == 7),
            )

    # ---- software-pipelined epilogue stages (issued >=1 step after deps) ----
    import collections
    sched = collections.defaultdict(list)

    def defer(step, fn):
        sched[step].append(fn)

    def emit_finals(sg, first_step, spread):
        """Stage the obars/transpose/final pipeline for supergroup sg.
        Each stage is issued `spread` steps after its producer so every
        instruction's deps are satisfied at issue time (no head-of-line
        blocking in the in-order engine queues)."""
        st = first_step
        ctx2 = {}

        def recs(oB):
            def f():
                rec = rcp.tile([P, 8], FP32, tag="rec", name=f"rec{sg}")
                nc.vector.reciprocal(rec[:], oB[:, :, D])
                ctx2["rec"] = rec
            return f

        def oc_copy(oB, h2):
            # raw (unnormalized) chain outputs -> SBUF bf16, one op per half
            def f():
                oc = obp.tile([P, 4, D], BF16, tag="ob", name=f"oc{sg}_{h2}")
                if h2 == 0:
                    nc.scalar.copy(oc[:], oB[:, h2 * 4:(h2 + 1) * 4, 0:D])
                else:
                    nc.vector.tensor_copy(oc[:], oB[:, h2 * 4:(h2 + 1) * 4, 0:D])
                ctx2[("oc", h2)] = oc
            return f

        def ot_mms(h2):
            # single matmul transposes all 4 bands: lhsT free dims (4, 32)
            # stack onto the 128 output partitions
            def f():
                oTps = psD.tile([P, P], FP32, tag="pd", name=f"otp{sg}_{h2}")
                nc.tensor.matmul(oTps[:], ctx2[("oc", h2)][:], ident[:],
                                 start=True, stop=True)
                ctx2[("otp", h2)] = oTps
            return f

        def ot_copy(h2):
            def f():
                oT = otp.tile([P, P], BF16, tag="ot", name=f"ot{sg}_{h2}")
                if h2 == 0:
                    nc.scalar.copy(oT[:], ctx2[("otp", h2)][:])
                else:
                    nc.vector.tensor_copy(oT[:], ctx2[("otp", h2)][:])
                ctx2[("ot", h2)] = oT
            return f

        def sp_move(h2):
            # band 3 lands at partition 96 which matmul lhsT cannot address;
            # transpose it again separately to a base-0 tile via PE
            def f():
                sp_ps = psD.tile([D, P], FP32, tag="pd", name=f"spp{sg}_{h2}")
                nc.tensor.matmul(sp_ps[:], ctx2[("oc", h2)][:, 3, :], ident[:],
                                 start=True, stop=True)
                sp = spp.tile([D, P], BF16, tag="sp", name=f"sp{sg}_{h2}")
                if h2 == 0:
                    nc.scalar.copy(sp[:], sp_ps[:])
                else:
                    nc.vector.tensor_copy(sp[:], sp_ps[:])
                ctx2[("sp", h2)] = sp
            return f

        def f_mm(h2, bd):
            def f():
                nt = sg * 8 + h2 * 4 + bd
                fps = psD.tile([P, C], FP32, tag="pd", name=f"F{nt}")
                if bd < 3:
                    nc.tensor.matmul(fps[:],
                                     ctx2[("ot", h2)][bd * D:(bd + 1) * D, :],
                                     wvr[bd * D:(bd + 1) * D, :],
                                     start=True, stop=True)
                else:
                    nc.tensor.matmul(fps[:], ctx2[("sp", h2)][:], wvr[0:D, :],
                                     start=True, stop=True)
                ctx2[("f", h2, bd)] = fps
            return f

        def f_out(h2, bd):
            def f():
                nt = sg * 8 + h2 * 4 + bd
                t = h2 * 4 + bd
                fps = ctx2[("f", h2, bd)]
                rec = ctx2["rec"]
                osb = fin.tile([P, C], FP32, tag="osb", name=f"osb{nt}")
                if bd % 2 == 0:
                    nc.scalar.activation(osb[:], fps[:],
                                         mybir.ActivationFunctionType.Copy,
                                         scale=rec[:, t:t + 1])
                    nc.gpsimd.tensor_add(osb[:], osb[:], x_half[:, nt, :])
                else:
                    nc.vector.tensor_scalar(osb[:], fps[:], rec[:, t:t + 1],
                                            None, Aop.mult)
                    nc.vector.tensor_add(osb[:], osb[:], x_half[:, nt, :])
                dq = nc.scalar if (sg == 1 and h2 == 1) else nc.sync
                dq.dma_start(out_d[nt * P:(nt + 1) * P, :], osb[:])
            return f

        oB = oBs[sg]
        if spread == 0:
            # dense tail: interleave both halves level-by-level so their
            # stage chains run concurrently on different engines
            defer(st, recs(oB))
            for h2 in range(2):
                defer(st, oc_copy(oB, h2))
            for h2 in range(2):
                defer(st, ot_mms(h2))
            for h2 in range(2):
                defer(st, ot_copy(h2))
            for h2 in range(2):
                defer(st, sp_move(h2))
            for bd in range(4):
                for h2 in range(2):
                    defer(st, f_mm(h2, bd))
            for bd in range(4):
                for h2 in range(2):
                    defer(st, f_out(h2, bd))
        else:
            defer(st, recs(oB))
            for h2 in range(2):
                b = st + (1 + h2 * 5) * spread
                defer(b, oc_copy(oB, h2))
                defer(b + spread, ot_mms(h2))
                defer(b + 2 * spread, ot_copy(h2))
                defer(b + 2 * spread, sp_move(h2))
                for bd in range(4):
                    defer(b + (3 + bd) * spread, f_mm(h2, bd))
                    defer(b + (4 + bd) * spread, f_out(h2, bd))

    # ---- main loop: flat over 64 beta/exp tiles, o-chains lag LAG steps ----
    oBs = [None, None]
    etiles = {}

    def o_step(s):
        sgp, j = divmod(s, MT)
        if j == 0:
            oBs[sgp] = psO.tile([P, 8, D + 1], FP32, tag="o", name=f"oB{sgp}")
        o_mms(sgp, j, oBs[sgp], etiles.pop(s))

    # x_half loads are only needed by the finals; put them on the sync queue
    # BEHIND the critical xt pieces (queues dispatch strictly in order)
    for piece in range(4):
        nc.sync.dma_start(
            x_half[:, piece * 4:(piece + 1) * 4, :],
            xh_src[:, piece * 4:(piece + 1) * 4, :],
        )

    # staged projection work: (kind, idx, mm_step); copy issues next step.
    # deadlines: q mg by step 4*mg-1; v mtg by step 4*mtg+LAG-1; k by 31
    stages = [
        ("v", 1, 4), ("q", 2, 5),
        ("v", 2, 7), ("q", 3, 9),
        ("k", 2, 10), ("v", 3, 11), ("q", 4, 13),
        ("k", 3, 14), ("v", 4, 15), ("q", 5, 17),
        ("v", 5, 19), ("q", 6, 21),
        ("v", 6, 23), ("q", 7, 25),
        ("v", 7, 27),
    ]
    for kind, idx, st in stages:
        def mk(kind, idx):
            def mm():
                if kind == "q":
                    etiles[("p", kind, idx)] = proj_mm(w_sb["f"], idx, "q")
                elif kind == "k":
                    etiles[("p", kind, idx)] = proj_mm(w_sb["g"], idx, "k")
                else:
                    etiles[("p", kind, idx)] = v_mm(idx)

            def cp():
                pp = etiles.pop(("p", kind, idx))
                if kind == "q":
                    proj_copy(pp, qt, idx)
                elif kind == "k":
                    proj_copy(pp, kt, idx)
                else:
                    v_copy(pp, idx)
            return mm, cp
        mm, cp = mk(kind, idx)
        defer(st, mm)
        defer(st + 1, cp)

    # finals for sg0 run spread through sg1; sg1's run densely at the end
    emit_finals_done = [False, False]

    NSTEPS = MT * SG
    for s in range(NSTEPS):
        sg, mt = divmod(s, MT)
        lane = pat[s % len(pat)]
        if lane == "A":
            et = ep.tile([P, SGW], BF16, tag="e", name=f"e{s}")
            etiles[s] = (et,)
            pb = psA.tile([P, SGW], FP32, tag="pa", name=f"pb{s}")
            for hf in range(2):
                nc.tensor.matmul(
                    pb[:, hf * 512:(hf + 1) * 512],
                    qt[:, mt * P:(mt + 1) * P],
                    kt[:, sg * SGW + hf * 512:sg * SGW + (hf + 1) * 512],
                    start=True, stop=True,
                )
            nc.scalar.activation(et[:], pb[:],
                                 mybir.ActivationFunctionType.Exp)
        else:
            ets = []
            for hf in range(2):
                eh = ep.tile([P, 512], BF16, tag="e", name=f"e{s}_{hf}")
                pb = psD.tile([P, 512], FP32, tag="pd", name=f"pb{s}_{hf}")
                nc.tensor.matmul(
                    pb[:],
                    qt[:, mt * P:(mt + 1) * P],
                    kt[:, sg * SGW + hf * 512:sg * SGW + (hf + 1) * 512],
                    start=True, stop=True,
                )
                nc.vector.tensor_scalar(
                    eh[:].bitcast(I16), pb[:],
                    EXP_S1, EXP_S2, Aop.mult, Aop.add)
                ets.append(eh)
            etiles[s] = tuple(ets)
        if s >= LAG:
            o_step(s - LAG)
            if s - LAG == MT - 1 and not emit_finals_done[0]:
                emit_finals(0, s + 1, 1)
                emit_finals_done[0] = True
        for fn in sched.pop(s, []):
            fn()
    # tail: trailing o-steps, then sg1 finals densely
    for s in range(NSTEPS - LAG, NSTEPS):
        o_step(s)
    emit_finals(1, NSTEPS, 0)
    for st in sorted(sched):
        for fn in sched.pop(st):
            fn()


def build_program():
    nc = bacc.Bacc(
        "TRN2",
        target_bir_lowering=False,
        debug=False,
        enable_asserts=False,
        num_devices=NCORES,
    )
    xh_d = nc.dram_tensor("xh", [NH, C], FP32, kind="ExternalInput").ap()
    xt_d = nc.dram_tensor("xt", [2, P, N], FP16, kind="ExternalInput").ap()
    wf_d = nc.dram_tensor("Wf16", [2, P, D], FP16, kind="ExternalInput").ap()
    wg_d = nc.dram_tensor("Wg16", [2, P, D], FP16, kind="ExternalInput").ap()
    wh_d = nc.dram_tensor("Wh16", [2, P, D], FP16, kind="ExternalInput").ap()
    wv_d = nc.dram_tensor("WvG", [P, C], BF16, kind="ExternalInput").ap()
    id_d = nc.dram_tensor("Ident", [P, P], BF16, kind="ExternalInput").ap()
    out_d = nc.dram_tensor("out", [NH, C], FP32, kind="ExternalOutput").ap()

    with tile.TileContext(nc) as tc:
        with ExitStack() as ctx:
            _body(ctx, tc, out_d, xh_d, xt_d, wf_d, wg_d, wh_d, wv_d, id_d)
    nc.compile()
    return nc


_CACHE = {}


def _get_program():
    if "nc" not in _CACHE:
        _CACHE["nc"] = build_program()
    return _CACHE["nc"]


def make_in_maps(inputs):
    x = np.ascontiguousarray(np.asarray(inputs["x"], np.float32)).reshape(B, N, C)
    gam = np.float32(np.asarray(inputs["gamma"], np.float32).reshape(()))
    w16 = {}
    for nm in ("Wf", "Wg", "Wh"):
        w = np.asarray(inputs[nm], np.float32).astype(np.float16)  # [256, 32]
        w16[nm] = np.ascontiguousarray(w.reshape(2, P, D))
    wv1 = (gam * np.asarray(inputs["Wv"], np.float32)).astype(ml_dtypes.bfloat16)
    wv = np.ascontiguousarray(np.tile(wv1, (4, 1)))
    ident = np.ascontiguousarray(np.eye(P, dtype=ml_dtypes.bfloat16))

    in_maps = []
    for c in range(NCORES):
        b, h = divmod(c, 2)
        if h == 0:
            xb = x[b]
        else:
            xb = np.concatenate([x[b, NH:], x[b, :NH]], axis=0)
        xt = np.ascontiguousarray(xb.T.astype(np.float16).reshape(2, P, N))
        in_maps.append(
            {
                "xh": np.ascontiguousarray(xb[:NH]),
                "xt": xt,
                "Wf16": w16["Wf"],
                "Wg16": w16["Wg"],
                "Wh16": w16["Wh"],
                "WvG": wv,
                "Ident": ident,
            }
        )
    return in_maps


def kernel(**inputs):
    global LAST_RESULTS
    nc = _get_program()
    in_maps = make_in_maps(inputs)
    res = run_bass_kernel_spmd(nc, in_maps, core_ids=list(range(NCORES)))
    LAST_RESULTS = res
    out = np.empty((B, N, C), np.float32)
    for c in range(NCORES):
        b, h = divmod(c, 2)
        out[b, h * NH:(h + 1) * NH] = res.results[c]["out"]
    return out.reshape(B, H, W, C)


# revision 33
# speedup vs baseline: 1.3735x; 1.0075x over previous
"""NonLocalBlock (self-attention over 64x64 image, C=256, D=32) on 8 trn2 cores.

Sharding: data-parallel over B=4 batches x 2-way split of the attention
rows (the `n` axis of beta[n, m]) => 8 cores, each computing a [2048, 256]
slice of the output. Each core receives its batch image pre-transposed
(and fp16-cast) by the host, rolled so its own 2048 rows come first,
plus its own half in natural layout for the residual. The host also
pre-casts the 1x1-conv weights and folds gamma into Wv.

Device math (per core, n = its 2048 key rows, m = all 4096 queries):
  logits[m, n] = q_m . k_n               PE, fp16, [128m x 512n] matmuls
  E[m, n] = exp(logits)                  ACT exact exp + DVE exp2 bit-trick
                                         (int16(l*128*log2e + 16250.875)
                                          bitcast to bf16, trunc-fitted)
  o[n, :] = sum_m E[m, n] v_aug[m, :]    PE, E stationary (33 cols/matmul),
                                         terms issued a few steps behind
                                         each exp tile; 8 chains share one
                                         PSUM zero-region (single start/stop)
  obar = o[:, 0:32] / o[:, 32]           DVE reciprocal + ACT/DVE scale, bf16
  oT = transpose(obar)                   PE (identity-moving), 32-row bands
  out[n, :] = oT.T @ (gamma Wv) + x      PE matmul + {ACT copy + Pool add |
                                         DVE fused add}, DMA out

Engine layout: ACT exp tiles are [128,1024] double-buffered in PSUM
(2x2 banks), DVE trick tiles are [128,512] double-buffered (2x1 banks),
o-chain accumulator 1 bank, final tiles 1 bank.
"""

from contextlib import ExitStack

import ml_dtypes
import numpy as np

import concourse.bass as bass
import concourse.tile as tile
from concourse import bacc, mybir
from concourse.bass_utils import run_bass_kernel_spmd

B, H, W, C = 4, 64, 64, 256
N = H * W            # 4096 pixels per image
D = 32               # reduced channel dim
NH = N // 2          # key rows owned by each core
P = 128
MT = N // P          # 32 query (m) tiles
NT = NH // P         # 16 n-tiles of 128 per core
SG = 2               # supergroups of 1024 n-columns
SGW = NH // SG       # 1024
FP32 = mybir.dt.float32
BF16 = mybir.dt.bfloat16
FP16 = mybir.dt.float16
I16 = mybir.dt.int16
NCORES = 8

# exp(l) ~= bf16-bitcast(int16(l * 128*log2(e) + 16250.875)); the int16
# convert truncates, constant fitted for that (max rel err 3.3%)
EXP_S1 = float(np.float32(128 * 1.4426950408889634))
EXP_S2 = 16250.875
Aop = mybir.AluOpType

LAST_RESULTS = None  # BassKernelResults of the most recent run (for test.py)

LAG = 4  # steps between an exp tile and its o-chain consumption


def _exp_pattern(n_act=19, n_dve=13):
    """Weighted round-robin ACT/DVE assignment for exp tiles (per 32)."""
    counts = {"A": float(n_act), "D": float(n_dve)}
    total = sum(counts.values())
    acc = dict.fromkeys(counts, 0.0)
    seq = []
    for _ in range(int(total)):
        for k in counts:
            acc[k] += counts[k] / total
        pick = max(acc, key=lambda k: acc[k])
        acc[pick] -= 1.0
        seq.append(pick)
    return seq


def _body(ctx, tc, out_d, xh_d, xt_d, wf_d, wg_d, wh_d, wv_d, id_d):
    nc = tc.nc
    const = ctx.enter_context(tc.tile_pool(name="const", bufs=1))
    big = ctx.enter_context(tc.tile_pool(name="big", bufs=1))
    ep = ctx.enter_context(tc.tile_pool(name="ep", bufs=10))
    obp = ctx.enter_context(tc.tile_pool(name="obp", bufs=4))
    otp = ctx.enter_context(tc.tile_pool(name="otp", bufs=4))
    spp = ctx.enter_context(tc.tile_pool(name="spp", bufs=2))
    ocp = ctx.enter_context(tc.tile_pool(name="ocp", bufs=2))
    rcp = ctx.enter_context(tc.tile_pool(name="rcp", bufs=4))
    fin = ctx.enter_context(tc.tile_pool(name="fin", bufs=8))
    psA = ctx.enter_context(tc.tile_pool(name="psA", bufs=2, space="PSUM"))
    psD = ctx.enter_context(tc.tile_pool(name="psD", bufs=3, space="PSUM"))
    psO = ctx.enter_context(tc.tile_pool(name="psO", bufs=1, space="PSUM"))

    # ---- tiny weights first (instant transfers) on the ACT HWDGE queue ----
    w_sb = {}
    for name, wd in (("f", wf_d), ("g", wg_d), ("h", wh_d)):
        wb = const.tile([P, 2, D], FP16, tag=f"w{name}")
        nc.scalar.dma_start(wb[:], wd.rearrange("c p d -> p c d"))
        w_sb[name] = wb
    wvr = const.tile([P, C], BF16)
    nc.scalar.dma_start(wvr[:], wv_d)
    ident = const.tile([P, P], BF16)
    nc.scalar.dma_start(ident[:], id_d)

    xt = big.tile([P, 2, N], FP16)  # xT: [c (2 chunks of 128), m]
    pieces = [(0, 512), (512, 1024), (1024, 2048), (2048, 3072), (3072, 4096)]
    for a, b in pieces:
        for ch in range(2):
            nc.sync.dma_start(xt[:, ch, a:b], xt_d[ch, :, a:b])
    x_half = big.tile([P, NT, C], FP32)
    qt = big.tile([D, N], FP16)            # q: [d, m]
    kt = big.tile([D, NH], FP16)           # k: [d, n] (own half only)
    v_sb = big.tile([P, MT, D + 1], BF16)  # v: [m, d | 1]
    nc.vector.memset(v_sb[:, :, D:D + 1], 1.0)

    def proj_mm(w, mg, nm):
        pp = psD.tile([D, 512], FP32, tag="pd", name=f"p{nm}{mg}")
        for ch in range(2):
            nc.tensor.matmul(
                pp[:], w[:, ch, :], xt[:, ch, mg * 512:(mg + 1) * 512],
                start=(ch == 0), stop=(ch == 1),
            )
        return pp

    def proj_copy(pp, dst, mg, on_act=False):
        if on_act:
            nc.scalar.copy(dst[:, mg * 512:(mg + 1) * 512], pp[:])
        else:
            nc.vector.tensor_copy(dst[:, mg * 512:(mg + 1) * 512], pp[:])

    def proj(w, dst, mg, nm, on_act=False):
        proj_copy(proj_mm(w, mg, nm), dst, mg, on_act)

    def v_mm(mtg):
        pv = psA.tile([P, 4, D], FP32, tag="pa", name=f"pv{mtg}")
        for j in range(4):
            mt = mtg * 4 + j
            for ch in range(2):
                nc.tensor.matmul(
                    pv[:, j, :], xt[:, ch, mt * P:(mt + 1) * P],
                    w_sb["h"][:, ch, :],
                    start=(ch == 0), stop=(ch == 1),
                )
        return pv

    def v_copy(pv, mtg, on_act=False):
        if on_act:
            nc.scalar.copy(v_sb[:, mtg * 4:(mtg + 1) * 4, 0:D], pv[:])
        else:
            nc.vector.tensor_copy(v_sb[:, mtg * 4:(mtg + 1) * 4, 0:D], pv[:])

    def v_batch(mtg, on_act=False):
        v_copy(v_mm(mtg), mtg, on_act)

    # PE p-state warmup: tiny matmuls on a memset tile (no DMA dependency)
    wsrc = big.tile([P, D], BF16, tag="wsrc")
    nc.vector.memset(wsrc[:], 0.25)
    warm = psA.tile([P, 64], FP32, tag="pa", name="warm")
    for _ in range(32):
        nc.tensor.matmul(
            warm[0:D, 0:D], wsrc[:], wsrc[:, 0:D],
            start=True, stop=True, skip_group_check=True,
        )
    nc.vector.tensor_copy(v_sb[0:D, 0, 0:D], warm[0:D, 0:D])  # keep it live

    # prologue projections (ACT is otherwise idle this early)
    pq0 = proj_mm(w_sb["f"], 0, "q")
    pk0 = proj_mm(w_sb["g"], 0, "k")
    proj_copy(pq0, qt, 0, on_act=True)
    proj_copy(pk0, kt, 0, on_act=False)
    pq1 = proj_mm(w_sb["f"], 1, "q")
    pk1 = proj_mm(w_sb["g"], 1, "k")
    proj_copy(pq1, qt, 1, on_act=True)
    proj_copy(pk1, kt, 1, on_act=False)
    v_batch(0, on_act=True)
    xh_src = xh_d.rearrange("(s p) c -> p s c", p=P)

    pat = _exp_pattern()

    def o_mms(sg, j, oB, ets):
        # one accumulation term (query tile j) for all 8 chains of the
        # supergroup; the chains share one PSUM zero-region, so only the
        # very first matmul starts it and the very last stops it (bytes
        # zero lazily on first touch)
        for t in range(8):
            if len(ets) == 1:
                esrc = ets[0][:, t * P:(t + 1) * P]
            else:
                esrc = ets[t // 4][:, (t % 4) * P:(t % 4 + 1) * P]
            nc.tensor.matmul(
                oB[:, t, :], esrc, v_sb[:, j, :],
                start=(j == 0 and t == 0), stop=(j == MT - 1 and t == 7),
            )

    # ---- software-pipelined epilogue stages (issued >=1 step after deps) ----
    import collections
    sched = collections.defaultdict(list)

    def defer(step, fn):
        sched[step].append(fn)

    def emit_finals(sg, first_step, spread):
        """Stage the obars/transpose/final pipeline for supergroup sg.
        Each stage is issued `spread` steps after its producer so every
        instruction's deps are satisfied at issue time (no head-of-line
        blocking in the in-order engine queues)."""
        st = first_step
        ctx2 = {}

        def recs(oB):
            def f():
                rec = rcp.tile([P, 8], FP32, tag="rec", name=f"rec{sg}")
                nc.vector.reciprocal(rec[:], oB[:, :, D])
                ctx2["rec"] = rec
            return f

        def oc_copy(oB, h2):
            # raw (unnormalized) chain outputs -> SBUF bf16, one op per half
            def f():
                oc = obp.tile([P, 4, D], BF16, tag="ob", name=f"oc{sg}_{h2}")
                if h2 == 0:
                    nc.scalar.copy(oc[:], oB[:, h2 * 4:(h2 + 1) * 4, 0:D])
                else:
                    nc.vector.tensor_copy(oc[:], oB[:, h2 * 4:(h2 + 1) * 4, 0:D])
                ctx2[("oc", h2)] = oc
            return f

        def ot_mms(h2):
            # single matmul transposes all 4 bands: lhsT free dims (4, 32)
            # stack onto the 128 output partitions
            def f():
                oTps = psD.tile([P, P], FP32, tag="pd", name=f"otp{sg}_{h2}")
                nc.tensor.matmul(oTps[:], ctx2[("oc", h2)][:], ident[:],
                                 start=True, stop=True)
                ctx2[("otp", h2)] = oTps
            return f

        def ot_copy(h2):
            def f():
                oT = otp.tile([P, P], BF16, tag="ot", name=f"ot{sg}_{h2}")
                if h2 == 0:
                    nc.scalar.copy(oT[:], ctx2[("otp", h2)][:])
                else:
                    nc.vector.tensor_copy(oT[:], ctx2[("otp", h2)][:])
                ctx2[("ot", h2)] = oT
            return f

        def sp_move(h2):
            # band 3 lands at partition 96 which matmul lhsT cannot address;
            # transpose it again separately to a base-0 tile via PE
            def f():
                sp_ps = psD.tile([D, P], FP32, tag="pd", name=f"spp{sg}_{h2}")
                nc.tensor.matmul(sp_ps[:], ctx2[("oc", h2)][:, 3, :], ident[:],
                                 start=True, stop=True)
                sp = spp.tile([D, P], BF16, tag="sp", name=f"sp{sg}_{h2}")
                if h2 == 0:
                    nc.scalar.copy(sp[:], sp_ps[:])
                else:
                    nc.vector.tensor_copy(sp[:], sp_ps[:])
                ctx2[("sp", h2)] = sp
            return f

        def f_mm(h2, bd):
            def f():
                nt = sg * 8 + h2 * 4 + bd
                fps = psD.tile([P, C], FP32, tag="pd", name=f"F{nt}")
                if bd < 3:
                    nc.tensor.matmul(fps[:],
                                     ctx2[("ot", h2)][bd * D:(bd + 1) * D, :],
                                     wvr[bd * D:(bd + 1) * D, :],
                                     start=True, stop=True)
                else:
                    nc.tensor.matmul(fps[:], ctx2[("sp", h2)][:], wvr[0:D, :],
                                     start=True, stop=True)
                ctx2[("f", h2, bd)] = fps
            return f

        def f_scale_add(h2, bd):
            def f():
                nt = sg * 8 + h2 * 4 + bd
                t = h2 * 4 + bd
                fps = ctx2[("f", h2, bd)]
                rec = ctx2["rec"]
                osb = fin.tile([P, C], FP32, tag="osb", name=f"osb{nt}")
                if bd % 2 == 0:
                    nc.scalar.activation(osb[:], fps[:],
                                         mybir.ActivationFunctionType.Copy,
                                         scale=rec[:, t:t + 1])
                    nc.gpsimd.tensor_add(osb[:], osb[:], x_half[:, nt, :])
                else:
                    nc.vector.tensor_scalar(osb[:], fps[:], rec[:, t:t + 1],
                                            None, Aop.mult)
                    nc.vector.tensor_add(osb[:], osb[:], x_half[:, nt, :])
                ctx2[("osb", h2, bd)] = osb
            return f

        def f_dma(h2, bd):
            def f():
                nt = sg * 8 + h2 * 4 + bd
                osb = ctx2[("osb", h2, bd)]
                dq = nc.scalar if (sg == 1 and h2 == 1) else nc.sync
                dq.dma_start(out_d[nt * P:(nt + 1) * P, :], osb[:])
            return f

        def f_out(h2, bd):
            def f():
                f_scale_add(h2, bd)()
                f_dma(h2, bd)()
            return f

        oB = oBs[sg]
        if spread == 0:
            # dense tail: interleave both halves level-by-level so their
            # stage chains run concurrently on different engines
            defer(st, recs(oB))
            for h2 in range(2):
                defer(st, oc_copy(oB, h2))
            for h2 in range(2):
                defer(st, ot_mms(h2))
            for h2 in range(2):
                defer(st, ot_copy(h2))
            for h2 in range(2):
                defer(st, sp_move(h2))
            for bd in range(4):
                for h2 in range(2):
                    defer(st, f_mm(h2, bd))
            for bd in range(4):
                for h2 in range(2):
                    defer(st, f_scale_add(h2, bd))
            for bd in range(4):
                for h2 in range(2):
                    defer(st, f_dma(h2, bd))
        else:
            defer(st, recs(oB))
            for h2 in range(2):
                b = st + (1 + h2 * 5) * spread
                defer(b, oc_copy(oB, h2))
                defer(b + spread, ot_mms(h2))
                defer(b + 2 * spread, ot_copy(h2))
                defer(b + 2 * spread, sp_move(h2))
                for bd in range(4):
                    defer(b + (3 + bd) * spread, f_mm(h2, bd))
                    defer(b + (4 + bd) * spread, f_out(h2, bd))

    # ---- main loop: flat over 64 beta/exp tiles, o-chains lag LAG steps ----
    oBs = [None, None]
    etiles = {}

    def o_step(s):
        sgp, j = divmod(s, MT)
        if j == 0:
            oBs[sgp] = psO.tile([P, 8, D + 1], FP32, tag="o", name=f"oB{sgp}")
        o_mms(sgp, j, oBs[sgp], etiles.pop(s))

    # x_half loads are only needed by the finals; put them on the sync queue
    # BEHIND the critical xt pieces (queues dispatch strictly in order)
    for piece in range(4):
        nc.sync.dma_start(
            x_half[:, piece * 4:(piece + 1) * 4, :],
            xh_src[:, piece * 4:(piece + 1) * 4, :],
        )

    # staged projection work: (kind, idx, mm_step); copy issues next step.
    # deadlines: q mg by step 4*mg-1; v mtg by step 4*mtg+LAG-1; k by 31
    stages = [
        ("v", 1, 4), ("q", 2, 5),
        ("v", 2, 7), ("q", 3, 9),
        ("k", 2, 10), ("v", 3, 11), ("q", 4, 13),
        ("k", 3, 14), ("v", 4, 15), ("q", 5, 17),
        ("v", 5, 19), ("q", 6, 21),
        ("v", 6, 23), ("q", 7, 25),
        ("v", 7, 27),
    ]
    for kind, idx, st in stages:
        def mk(kind, idx):
            def mm():
                if kind == "q":
                    etiles[("p", kind, idx)] = proj_mm(w_sb["f"], idx, "q")
                elif kind == "k":
                    etiles[("p", kind, idx)] = proj_mm(w_sb["g"], idx, "k")
                else:
                    etiles[("p", kind, idx)] = v_mm(idx)

            def cp():
                pp = etiles.pop(("p", kind, idx))
                if kind == "q":
                    proj_copy(pp, qt, idx)
                elif kind == "k":
                    proj_copy(pp, kt, idx)
                else:
                    v_copy(pp, idx, on_act=True)
            return mm, cp
        mm, cp = mk(kind, idx)
        defer(st, mm)
        defer(st + 1, cp)

    # finals for sg0 run spread through sg1; sg1's run densely at the end
    emit_finals_done = [False, False]

    NSTEPS = MT * SG
    for s in range(NSTEPS):
        sg, mt = divmod(s, MT)
        lane = pat[s % len(pat)]
        if lane == "A":
            et = ep.tile([P, SGW], BF16, tag="e", name=f"e{s}")
            etiles[s] = (et,)
            pb = psA.tile([P, SGW], FP32, tag="pa", name=f"pb{s}")
            for hf in range(2):
                nc.tensor.matmul(
                    pb[:, hf * 512:(hf + 1) * 512],
                    qt[:, mt * P:(mt + 1) * P],
                    kt[:, sg * SGW + hf * 512:sg * SGW + (hf + 1) * 512],
                    start=True, stop=True,
                )
            nc.scalar.activation(et[:], pb[:],
                                 mybir.ActivationFunctionType.Exp)
        else:
            ets = []
            for hf in range(2):
                eh = ep.tile([P, 512], BF16, tag="e", name=f"e{s}_{hf}")
                pb = psD.tile([P, 512], FP32, tag="pd", name=f"pb{s}_{hf}")
                nc.tensor.matmul(
                    pb[:],
                    qt[:, mt * P:(mt + 1) * P],
                    kt[:, sg * SGW + hf * 512:sg * SGW + (hf + 1) * 512],
                    start=True, stop=True,
                )
                nc.vector.tensor_scalar(
                    eh[:].bitcast(I16), pb[:],
                    EXP_S1, EXP_S2, Aop.mult, Aop.add)
                ets.append(eh)
            etiles[s] = tuple(ets)
        if s >= LAG:
            o_step(s - LAG)
            if s - LAG == MT - 1 and not emit_finals_done[0]:
                emit_finals(0, s + 1, 1)
                emit_finals_done[0] = True
        for fn in sched.pop(s, []):
            fn()
    # tail: trailing o-steps, then sg1 finals densely
    for s in range(NSTEPS - LAG, NSTEPS):
        o_step(s)
    emit_finals(1, NSTEPS, 0)
    for st in sorted(sched):
        for fn in sched.pop(st):
            fn()


def build_program():
    nc = bacc.Bacc(
        "TRN2",
        target_bir_lowering=False,
        debug=False,
        enable_asserts=False,
        num_devices=NCORES,
    )
    xh_d = nc.dram_tensor("xh", [NH, C], FP32, kind="ExternalInput").ap()
    xt_d = nc.dram_tensor("xt", [2, P, N], FP16, kind="ExternalInput").ap()
    wf_d = nc.dram_tensor("Wf16", [2, P, D], FP16, kind="ExternalInput").ap()
    wg_d = nc.dram_tensor("Wg16", [2, P, D], FP16, kind="ExternalInput").ap()
    wh_d = nc.dram_tensor("Wh16", [2, P, D], FP16, kind="ExternalInput").ap()
    wv_d = nc.dram_tensor("WvG", [P, C], BF16, kind="ExternalInput").ap()
    id_d = nc.dram_tensor("Ident", [P, P], BF16, kind="ExternalInput").ap()
    out_d = nc.dram_tensor("out", [NH, C], FP32, kind="ExternalOutput").ap()

    with tile.TileContext(nc) as tc:
        with ExitStack() as ctx:
            _body(ctx, tc, out_d, xh_d, xt_d, wf_d, wg_d, wh_d, wv_d, id_d)
    nc.compile()
    return nc


_CACHE = {}


def _get_program():
    if "nc" not in _CACHE:
        _CACHE["nc"] = build_program()
    return _CACHE["nc"]


def make_in_maps(inputs):
    x = np.ascontiguousarray(np.asarray(inputs["x"], np.float32)).reshape(B, N, C)
    gam = np.float32(np.asarray(inputs["gamma"], np.float32).reshape(()))
    w16 = {}
    for nm in ("Wf", "Wg", "Wh"):
        w = np.asarray(inputs[nm], np.float32).astype(np.float16)  # [256, 32]
        w16[nm] = np.ascontiguousarray(w.reshape(2, P, D))
    wv1 = (gam * np.asarray(inputs["Wv"], np.float32)).astype(ml_dtypes.bfloat16)
    wv = np.ascontiguousarray(np.tile(wv1, (4, 1)))
    ident = np.ascontiguousarray(np.eye(P, dtype=ml_dtypes.bfloat16))

    in_maps = []
    for c in range(NCORES):
        b, h = divmod(c, 2)
        if h == 0:
            xb = x[b]
        else:
            xb = np.concatenate([x[b, NH:], x[b, :NH]], axis=0)
        xt = np.ascontiguousarray(xb.T.astype(np.float16).reshape(2, P, N))
        in_maps.append(
            {
                "xh": np.ascontiguousarray(xb[:NH]),
                "xt": xt,
                "Wf16": w16["Wf"],
                "Wg16": w16["Wg"],
                "Wh16": w16["Wh"],
                "WvG": wv,
                "Ident": ident,
            }
        )
    return in_maps


def kernel(**inputs):
    global LAST_RESULTS
    nc = _get_program()
    in_maps = make_in_maps(inputs)
    res = run_bass_kernel_spmd(nc, in_maps, core_ids=list(range(NCORES)))
    LAST_RESULTS = res
    out = np.empty((B, N, C), np.float32)
    for c in range(NCORES):
        b, h = divmod(c, 2)
        out[b, h * NH:(h + 1) * NH] = res.results[c]["out"]
    return out.reshape(B, H, W, C)


# revision 39
# speedup vs baseline: 1.3762x; 1.0020x over previous
"""NonLocalBlock (self-attention over 64x64 image, C=256, D=32) on 8 trn2 cores.

Sharding: data-parallel over B=4 batches x 2-way split of the attention
rows (the `n` axis of beta[n, m]) => 8 cores, each computing a [2048, 256]
slice of the output. Each core receives its batch image pre-transposed
(and fp16-cast) by the host, rolled so its own 2048 rows come first,
plus its own half in natural layout for the residual. The host also
pre-casts the 1x1-conv weights and folds gamma into Wv.

Device math (per core, n = its 2048 key rows, m = all 4096 queries):
  logits[m, n] = q_m . k_n               PE, fp16, [128m x 512n] matmuls
  E[m, n] = exp(logits)                  ACT exact exp + DVE exp2 bit-trick
                                         (int16(l*128*log2e + 16250.875)
                                          bitcast to bf16, trunc-fitted)
  o[n, :] = sum_m E[m, n] v_aug[m, :]    PE, E stationary (33 cols/matmul),
                                         terms issued a few steps behind
                                         each exp tile; 8 chains share one
                                         PSUM zero-region (single start/stop)
  obar = o[:, 0:32] / o[:, 32]           DVE reciprocal + ACT/DVE scale, bf16
  oT = transpose(obar)                   PE (identity-moving), 32-row bands
  out[n, :] = oT.T @ (gamma Wv) + x      PE matmul + {ACT copy + Pool add |
                                         DVE fused add}, DMA out

Engine layout: ACT exp tiles are [128,1024] double-buffered in PSUM
(2x2 banks), DVE trick tiles are [128,512] double-buffered (2x1 banks),
o-chain accumulator 1 bank, final tiles 1 bank.
"""

from contextlib import ExitStack

import ml_dtypes
import numpy as np

import concourse.bass as bass
import concourse.tile as tile
from concourse import bacc, mybir
from concourse.bass_utils import run_bass_kernel_spmd

B, H, W, C = 4, 64, 64, 256
N = H * W            # 4096 pixels per image
D = 32               # reduced channel dim
NH = N // 2          # key rows owned by each core
P = 128
MT = N // P          # 32 query (m) tiles
NT = NH // P         # 16 n-tiles of 128 per core
SG = 2               # supergroups of 1024 n-columns
SGW = NH // SG       # 1024
FP32 = mybir.dt.float32
BF16 = mybir.dt.bfloat16
FP16 = mybir.dt.float16
I16 = mybir.dt.int16
NCORES = 8

# exp(l) ~= bf16-bitcast(int16(l * 128*log2(e) + 16250.875)); the int16
# convert truncates, constant fitted for that (max rel err 3.3%)
EXP_S1 = float(np.float32(128 * 1.4426950408889634))
EXP_S2 = 16250.875
Aop = mybir.AluOpType

LAST_RESULTS = None  # BassKernelResults of the most recent run (for test.py)

LAG = 4  # steps between an exp tile and its o-chain consumption


def _exp_pattern(n_act=19, n_dve=13):
    """Weighted round-robin ACT/DVE assignment for exp tiles (per 32)."""
    counts = {"A": float(n_act), "D": float(n_dve)}
    total = sum(counts.values())
    acc = dict.fromkeys(counts, 0.0)
    seq = []
    for _ in range(int(total)):
        for k in counts:
            acc[k] += counts[k] / total
        pick = max(acc, key=lambda k: acc[k])
        acc[pick] -= 1.0
        seq.append(pick)
    return seq


def _body(ctx, tc, out_d, xh_d, xt_d, wf_d, wg_d, wh_d, wv_d, id_d):
    nc = tc.nc
    const = ctx.enter_context(tc.tile_pool(name="const", bufs=1))
    big = ctx.enter_context(tc.tile_pool(name="big", bufs=1))
    ep = ctx.enter_context(tc.tile_pool(name="ep", bufs=10))
    obp = ctx.enter_context(tc.tile_pool(name="obp", bufs=4))
    otp = ctx.enter_context(tc.tile_pool(name="otp", bufs=4))
    spp = ctx.enter_context(tc.tile_pool(name="spp", bufs=2))
    ocp = ctx.enter_context(tc.tile_pool(name="ocp", bufs=2))
    rcp = ctx.enter_context(tc.tile_pool(name="rcp", bufs=4))
    fin = ctx.enter_context(tc.tile_pool(name="fin", bufs=16))
    psA = ctx.enter_context(tc.tile_pool(name="psA", bufs=2, space="PSUM"))
    psD = ctx.enter_context(tc.tile_pool(name="psD", bufs=3, space="PSUM"))
    psO = ctx.enter_context(tc.tile_pool(name="psO", bufs=1, space="PSUM"))

    # ---- tiny weights first (instant transfers) on the ACT HWDGE queue ----
    w_sb = {}
    for name, wd in (("f", wf_d), ("g", wg_d), ("h", wh_d)):
        wb = const.tile([P, 2, D], FP16, tag=f"w{name}")
        nc.scalar.dma_start(wb[:], wd.rearrange("c p d -> p c d"))
        w_sb[name] = wb
    wvr = const.tile([P, C], BF16)
    nc.scalar.dma_start(wvr[:], wv_d)
    ident = const.tile([P, P], BF16)
    nc.scalar.dma_start(ident[:], id_d)

    xt = big.tile([P, 2, N], FP16)  # xT: [c (2 chunks of 128), m]
    pieces = [(0, 512), (512, 1024), (1024, 2048), (2048, 3072), (3072, 4096)]
    for a, b in pieces:
        for ch in range(2):
            nc.sync.dma_start(xt[:, ch, a:b], xt_d[ch, :, a:b])
    x_half = big.tile([P, NT, C], FP32)
    qt = big.tile([D, N], FP16)            # q: [d, m]
    kt = big.tile([D, NH], FP16)           # k: [d, n] (own half only)
    v_sb = big.tile([P, MT, D + 1], BF16)  # v: [m, d | 1]
    nc.vector.memset(v_sb[:, :, D:D + 1], 1.0)

    def proj_mm(w, mg, nm):
        pp = psD.tile([D, 512], FP32, tag="pd", name=f"p{nm}{mg}")
        for ch in range(2):
            nc.tensor.matmul(
                pp[:], w[:, ch, :], xt[:, ch, mg * 512:(mg + 1) * 512],
                start=(ch == 0), stop=(ch == 1),
            )
        return pp

    def proj_copy(pp, dst, mg, on_act=False):
        if on_act:
            nc.scalar.copy(dst[:, mg * 512:(mg + 1) * 512], pp[:])
        else:
            nc.vector.tensor_copy(dst[:, mg * 512:(mg + 1) * 512], pp[:])

    def proj(w, dst, mg, nm, on_act=False):
        proj_copy(proj_mm(w, mg, nm), dst, mg, on_act)

    def v_mm(mtg):
        pv = psA.tile([P, 4, D], FP32, tag="pa", name=f"pv{mtg}")
        for j in range(4):
            mt = mtg * 4 + j
            for ch in range(2):
                nc.tensor.matmul(
                    pv[:, j, :], xt[:, ch, mt * P:(mt + 1) * P],
                    w_sb["h"][:, ch, :],
                    start=(ch == 0), stop=(ch == 1),
                )
        return pv

    def v_copy(pv, mtg, on_act=False):
        if on_act:
            nc.scalar.copy(v_sb[:, mtg * 4:(mtg + 1) * 4, 0:D], pv[:])
        else:
            nc.vector.tensor_copy(v_sb[:, mtg * 4:(mtg + 1) * 4, 0:D], pv[:])

    def v_batch(mtg, on_act=False):
        v_copy(v_mm(mtg), mtg, on_act)

    # PE p-state warmup: tiny matmuls on a memset tile (no DMA dependency)
    wsrc = big.tile([P, D], BF16, tag="wsrc")
    nc.vector.memset(wsrc[:], 0.25)
    warm = psA.tile([P, 64], FP32, tag="pa", name="warm")
    for _ in range(32):
        nc.tensor.matmul(
            warm[0:D, 0:D], wsrc[:], wsrc[:, 0:D],
            start=True, stop=True, skip_group_check=True,
        )
    nc.vector.tensor_copy(v_sb[0:D, 0, 0:D], warm[0:D, 0:D])  # keep it live

    # prologue projections (ACT is otherwise idle this early)
    pq0 = proj_mm(w_sb["f"], 0, "q")
    pk0 = proj_mm(w_sb["g"], 0, "k")
    proj_copy(pq0, qt, 0, on_act=True)
    proj_copy(pk0, kt, 0, on_act=False)
    pq1 = proj_mm(w_sb["f"], 1, "q")
    pk1 = proj_mm(w_sb["g"], 1, "k")
    proj_copy(pq1, qt, 1, on_act=True)
    proj_copy(pk1, kt, 1, on_act=False)
    v_batch(0, on_act=True)
    xh_src = xh_d.rearrange("(s p) c -> p s c", p=P)

    pat0 = _exp_pattern(19, 13)   # sg0: DVE busy with staged proj copies
    pat1 = _exp_pattern(19, 13)   # sg1: DVE freer

    def o_mms(sg, j, oB, ets):
        # one accumulation term (query tile j) for all 8 chains of the
        # supergroup; the chains share one PSUM zero-region, so only the
        # very first matmul starts it and the very last stops it (bytes
        # zero lazily on first touch)
        for t in range(8):
            if len(ets) == 1:
                esrc = ets[0][:, t * P:(t + 1) * P]
            else:
                esrc = ets[t // 4][:, (t % 4) * P:(t % 4 + 1) * P]
            nc.tensor.matmul(
                oB[:, t, :], esrc, v_sb[:, j, :],
                start=(j == 0 and t == 0), stop=(j == MT - 1 and t == 7),
            )

    # ---- software-pipelined epilogue stages (issued >=1 step after deps) ----
    import collections
    sched = collections.defaultdict(list)

    def defer(step, fn):
        sched[step].append(fn)

    def emit_finals(sg, first_step, spread):
        """Stage the obars/transpose/final pipeline for supergroup sg.
        Each stage is issued `spread` steps after its producer so every
        instruction's deps are satisfied at issue time (no head-of-line
        blocking in the in-order engine queues)."""
        st = first_step
        ctx2 = {}

        def recs(oB):
            def f():
                rec = rcp.tile([P, 8], FP32, tag="rec", name=f"rec{sg}")
                nc.vector.reciprocal(rec[:], oB[:, :, D])
                ctx2["rec"] = rec
            return f

        def oc_copy(oB, h2):
            # raw (unnormalized) chain outputs -> SBUF bf16, one op per half
            def f():
                oc = obp.tile([P, 4, D], BF16, tag="ob", name=f"oc{sg}_{h2}")
                if h2 == 0:
                    nc.scalar.copy(oc[:], oB[:, h2 * 4:(h2 + 1) * 4, 0:D])
                else:
                    nc.vector.tensor_copy(oc[:], oB[:, h2 * 4:(h2 + 1) * 4, 0:D])
                ctx2[("oc", h2)] = oc
            return f

        def ot_mms(h2):
            # single matmul transposes all 4 bands: lhsT free dims (4, 32)
            # stack onto the 128 output partitions
            def f():
                oTps = psD.tile([P, P], FP32, tag="pd", name=f"otp{sg}_{h2}")
                nc.tensor.matmul(oTps[:], ctx2[("oc", h2)][:], ident[:],
                                 start=True, stop=True)
                ctx2[("otp", h2)] = oTps
            return f

        def ot_copy(h2):
            def f():
                oT = otp.tile([P, P], BF16, tag="ot", name=f"ot{sg}_{h2}")
                if h2 == 0:
                    nc.scalar.copy(oT[:], ctx2[("otp", h2)][:])
                else:
                    nc.vector.tensor_copy(oT[:], ctx2[("otp", h2)][:])
                ctx2[("ot", h2)] = oT
            return f

        def sp_move(h2):
            # band 3 lands at partition 96 which matmul lhsT cannot address;
            # transpose it again separately to a base-0 tile via PE
            def f():
                sp_ps = psD.tile([D, P], FP32, tag="pd", name=f"spp{sg}_{h2}")
                nc.tensor.matmul(sp_ps[:], ctx2[("oc", h2)][:, 3, :], ident[:],
                                 start=True, stop=True)
                sp = spp.tile([D, P], BF16, tag="sp", name=f"sp{sg}_{h2}")
                if h2 == 0:
                    nc.scalar.copy(sp[:], sp_ps[:])
                else:
                    nc.vector.tensor_copy(sp[:], sp_ps[:])
                ctx2[("sp", h2)] = sp
            return f

        def f_mm(h2, bd):
            def f():
                nt = sg * 8 + h2 * 4 + bd
                # tail finals can also use the (then idle) ACT-lane slots
                fpool, ftag = (psA, "pa") if (sg == 1 and bd % 2 == 1) else (psD, "pd")
                fps = fpool.tile([P, C], FP32, tag=ftag, name=f"F{nt}")
                if bd < 3:
                    nc.tensor.matmul(fps[:],
                                     ctx2[("ot", h2)][bd * D:(bd + 1) * D, :],
                                     wvr[bd * D:(bd + 1) * D, :],
                                     start=True, stop=True)
                else:
                    nc.tensor.matmul(fps[:], ctx2[("sp", h2)][:], wvr[0:D, :],
                                     start=True, stop=True)
                ctx2[("f", h2, bd)] = fps
            return f

        def f_scale_add(h2, bd):
            def f():
                nt = sg * 8 + h2 * 4 + bd
                t = h2 * 4 + bd
                fps = ctx2[("f", h2, bd)]
                rec = ctx2["rec"]
                osb = fin.tile([P, C], FP32, tag="osb", name=f"osb{nt}")
                if bd % 2 == 0:
                    nc.scalar.activation(osb[:], fps[:],
                                         mybir.ActivationFunctionType.Copy,
                                         scale=rec[:, t:t + 1])
                    nc.gpsimd.tensor_add(osb[:], osb[:], x_half[:, nt, :])
                else:
                    nc.vector.tensor_scalar(osb[:], fps[:], rec[:, t:t + 1],
                                            None, Aop.mult)
                    nc.vector.tensor_add(osb[:], osb[:], x_half[:, nt, :])
                ctx2[("osb", h2, bd)] = osb
            return f

        def f_dma(h2, bd):
            def f():
                nt = sg * 8 + h2 * 4 + bd
                osb = ctx2[("osb", h2, bd)]
                dq = nc.scalar if (sg == 1 and h2 == 1) else nc.sync
                dq.dma_start(out_d[nt * P:(nt + 1) * P, :], osb[:])
            return f

        def f_out(h2, bd):
            def f():
                f_scale_add(h2, bd)()
                f_dma(h2, bd)()
            return f

        oB = oBs[sg]
        if spread == 0:
            # dense tail: interleave both halves level-by-level so their
            # stage chains run concurrently on different engines
            defer(st, recs(oB))
            for h2 in range(2):
                defer(st, oc_copy(oB, h2))
            for h2 in range(2):
                defer(st, ot_mms(h2))
            for h2 in range(2):
                defer(st, ot_copy(h2))
            for h2 in range(2):
                defer(st, sp_move(h2))
            for bd in range(4):
                for h2 in range(2):
                    defer(st, f_mm(h2, bd))
            for bd in range(4):
                for h2 in range(2):
                    defer(st, f_scale_add(h2, bd))
            for bd in range(4):
                for h2 in range(2):
                    defer(st, f_dma(h2, bd))
        else:
            defer(st, recs(oB))
            for h2 in range(2):
                b = st + (1 + h2 * 5) * spread
                defer(b, oc_copy(oB, h2))
                defer(b + spread, ot_mms(h2))
                defer(b + 2 * spread, ot_copy(h2))
                defer(b + 2 * spread, sp_move(h2))
                for bd in range(4):
                    defer(b + (3 + bd) * spread, f_mm(h2, bd))
                    defer(b + (4 + bd) * spread, f_out(h2, bd))

    # ---- main loop: flat over 64 beta/exp tiles, o-chains lag LAG steps ----
    oBs = [None, None]
    etiles = {}

    def o_step(s):
        sgp, j = divmod(s, MT)
        if j == 0:
            oBs[sgp] = psO.tile([P, 8, D + 1], FP32, tag="o", name=f"oB{sgp}")
        o_mms(sgp, j, oBs[sgp], etiles.pop(s))

    # x_half loads are only needed by the finals; put them on the sync queue
    # BEHIND the critical xt pieces (queues dispatch strictly in order)
    for piece in range(4):
        nc.sync.dma_start(
            x_half[:, piece * 4:(piece + 1) * 4, :],
            xh_src[:, piece * 4:(piece + 1) * 4, :],
        )

    # staged projection work: (kind, idx, mm_step); copy issues next step.
    # deadlines: q mg by step 4*mg-1; v mtg by step 4*mtg+LAG-1; k by 31
    stages = [
        ("v", 1, 4), ("q", 2, 5),
        ("v", 2, 7), ("q", 3, 9),
        ("k", 2, 10), ("v", 3, 11), ("q", 4, 13),
        ("k", 3, 14), ("v", 4, 15), ("q", 5, 17),
        ("v", 5, 19), ("q", 6, 21),
        ("v", 6, 23), ("q", 7, 25),
        ("v", 7, 27),
    ]
    for kind, idx, st in stages:
        def mk(kind, idx):
            def mm():
                if kind == "q":
                    etiles[("p", kind, idx)] = proj_mm(w_sb["f"], idx, "q")
                elif kind == "k":
                    etiles[("p", kind, idx)] = proj_mm(w_sb["g"], idx, "k")
                else:
                    etiles[("p", kind, idx)] = v_mm(idx)

            def cp():
                pp = etiles.pop(("p", kind, idx))
                if kind == "q":
                    proj_copy(pp, qt, idx)
                elif kind == "k":
                    proj_copy(pp, kt, idx)
                else:
                    v_copy(pp, idx, on_act=True)
            return mm, cp
        mm, cp = mk(kind, idx)
        defer(st, mm)
        defer(st + 1, cp)

    # finals for sg0 run spread through sg1; sg1's run densely at the end
    emit_finals_done = [False, False]

    NSTEPS = MT * SG
    for s in range(NSTEPS):
        sg, mt = divmod(s, MT)
        lane = (pat0 if sg == 0 else pat1)[s % 32]
        if lane == "A":
            et = ep.tile([P, SGW], BF16, tag="e", name=f"e{s}")
            etiles[s] = (et,)
            pb = psA.tile([P, SGW], FP32, tag="pa", name=f"pb{s}")
            for hf in range(2):
                nc.tensor.matmul(
                    pb[:, hf * 512:(hf + 1) * 512],
                    qt[:, mt * P:(mt + 1) * P],
                    kt[:, sg * SGW + hf * 512:sg * SGW + (hf + 1) * 512],
                    start=True, stop=True,
                )
            nc.scalar.activation(et[:], pb[:],
                                 mybir.ActivationFunctionType.Exp)
        else:
            ets = []
            for hf in range(2):
                eh = ep.tile([P, 512], BF16, tag="e", name=f"e{s}_{hf}")
                pb = psD.tile([P, 512], FP32, tag="pd", name=f"pb{s}_{hf}")
                nc.tensor.matmul(
                    pb[:],
                    qt[:, mt * P:(mt + 1) * P],
                    kt[:, sg * SGW + hf * 512:sg * SGW + (hf + 1) * 512],
                    start=True, stop=True,
                )
                nc.vector.tensor_scalar(
                    eh[:].bitcast(I16), pb[:],
                    EXP_S1, EXP_S2, Aop.mult, Aop.add)
                ets.append(eh)
            etiles[s] = tuple(ets)
        if s >= LAG:
            o_step(s - LAG)
            if s - LAG == MT - 1 and not emit_finals_done[0]:
                emit_finals(0, s + 1, 1)
                emit_finals_done[0] = True
        for fn in sched.pop(s, []):
            fn()
    # tail: trailing o-steps, then sg1 finals densely
    for s in range(NSTEPS - LAG, NSTEPS):
        o_step(s)
    emit_finals(1, NSTEPS, 0)
    for st in sorted(sched):
        for fn in sched.pop(st):
            fn()


def build_program():
    nc = bacc.Bacc(
        "TRN2",
        target_bir_lowering=False,
        debug=False,
        enable_asserts=False,
        num_devices=NCORES,
    )
    xh_d = nc.dram_tensor("xh", [NH, C], FP32, kind="ExternalInput").ap()
    xt_d = nc.dram_tensor("xt", [2, P, N], FP16, kind="ExternalInput").ap()
    wf_d = nc.dram_tensor("Wf16", [2, P, D], FP16, kind="ExternalInput").ap()
    wg_d = nc.dram_tensor("Wg16", [2, P, D], FP16, kind="ExternalInput").ap()
    wh_d = nc.dram_tensor("Wh16", [2, P, D], FP16, kind="ExternalInput").ap()
    wv_d = nc.dram_tensor("WvG", [P, C], BF16, kind="ExternalInput").ap()
    id_d = nc.dram_tensor("Ident", [P, P], BF16, kind="ExternalInput").ap()
    out_d = nc.dram_tensor("out", [NH, C], FP32, kind="ExternalOutput").ap()

    with tile.TileContext(nc) as tc:
        with ExitStack() as ctx:
            _body(ctx, tc, out_d, xh_d, xt_d, wf_d, wg_d, wh_d, wv_d, id_d)
    nc.compile()
    return nc


_CACHE = {}


def _get_program():
    if "nc" not in _CACHE:
        _CACHE["nc"] = build_program()
    return _CACHE["nc"]


def make_in_maps(inputs):
    x = np.ascontiguousarray(np.asarray(inputs["x"], np.float32)).reshape(B, N, C)
    gam = np.float32(np.asarray(inputs["gamma"], np.float32).reshape(()))
    w16 = {}
    for nm in ("Wf", "Wg", "Wh"):
        w = np.asarray(inputs[nm], np.float32).astype(np.float16)  # [256, 32]
        w16[nm] = np.ascontiguousarray(w.reshape(2, P, D))
    wv1 = (gam * np.asarray(inputs["Wv"], np.float32)).astype(ml_dtypes.bfloat16)
    wv = np.ascontiguousarray(np.tile(wv1, (4, 1)))
    ident = np.ascontiguousarray(np.eye(P, dtype=ml_dtypes.bfloat16))

    in_maps = []
    for c in range(NCORES):
        b, h = divmod(c, 2)
        if h == 0:
            xb = x[b]
        else:
            xb = np.concatenate([x[b, NH:], x[b, :NH]], axis=0)
        xt = np.ascontiguousarray(xb.T.astype(np.float16).reshape(2, P, N))
        in_maps.append(
            {
                "xh": np.ascontiguousarray(xb[:NH]),
                "xt": xt,
                "Wf16": w16["Wf"],
                "Wg16": w16["Wg"],
                "Wh16": w16["Wh"],
                "WvG": wv,
                "Ident": ident,
            }
        )
    return in_maps


def kernel(**inputs):
    global LAST_RESULTS
    nc = _get_program()
    in_maps = make_in_maps(inputs)
    res = run_bass_kernel_spmd(nc, in_maps, core_ids=list(range(NCORES)))
    LAST_RESULTS = res
    out = np.empty((B, N, C), np.float32)
    for c in range(NCORES):
        b, h = divmod(c, 2)
        out[b, h * NH:(h + 1) * NH] = res.results[c]["out"]
    return out.reshape(B, H, W, C)


# revision 48
# speedup vs baseline: 1.4158x; 1.0288x over previous
"""NonLocalBlock (self-attention over 64x64 image, C=256, D=32) on 8 trn2 cores.

Sharding: data-parallel over B=4 batches x 2-way split of the attention
rows (the `n` axis of beta[n, m]) => 8 cores, each computing a [2048, 256]
slice of the output. Each core receives its batch image pre-transposed
(and fp16-cast) by the host, rolled so its own 2048 rows come first,
plus its own half in natural layout for the residual. The host also
pre-casts the 1x1-conv weights and folds gamma into Wv.

Device math (per core, n = its 2048 key rows, m = all 4096 queries):
  logits[m, n] = q_m . k_n               PE, fp16, [128m x 512n] matmuls
  E[m, n] = exp(logits)                  ACT exact exp + DVE exp2 bit-trick
                                         (int16(l*128*log2e + 16250.875)
                                          bitcast to bf16, trunc-fitted)
  o[n, :] = sum_m E[m, n] v_aug[m, :]    PE, E stationary (33 cols/matmul),
                                         terms issued a few steps behind
                                         each exp tile; 8 chains share one
                                         PSUM zero-region (single start/stop)
  obar = o[:, 0:32] / o[:, 32]           DVE reciprocal + ACT/DVE scale, bf16
  oT = transpose(obar)                   PE (identity-moving), 32-row bands
  out[n, :] = oT.T @ (gamma Wv) + x      PE matmul + {ACT copy + Pool add |
                                         DVE fused add}, DMA out

Engine layout: ACT exp tiles are [128,1024] double-buffered in PSUM
(2x2 banks), DVE trick tiles are [128,512] double-buffered (2x1 banks),
o-chain accumulator 1 bank, final tiles 1 bank.
"""

from contextlib import ExitStack

import ml_dtypes
import numpy as np

import concourse.bass as bass
import concourse.tile as tile
from concourse import bacc, mybir
from concourse.bass_utils import run_bass_kernel_spmd

B, H, W, C = 4, 64, 64, 256
N = H * W            # 4096 pixels per image
D = 32               # reduced channel dim
NH = N // 2          # key rows owned by each core
P = 128
MT = N // P          # 32 query (m) tiles
NT = NH // P         # 16 n-tiles of 128 per core
SG = 2               # supergroups of 1024 n-columns
SGW = NH // SG       # 1024
FP32 = mybir.dt.float32
BF16 = mybir.dt.bfloat16
FP16 = mybir.dt.float16
I16 = mybir.dt.int16
NCORES = 8

# exp(l) ~= bf16-bitcast(int16(l * 128*log2(e) + 16250.875)); the int16
# convert truncates, constant fitted for that (max rel err 3.3%)
EXP_S1 = float(np.float32(128 * 1.4426950408889634))
EXP_S2 = 16250.875
Aop = mybir.AluOpType

LAST_RESULTS = None  # BassKernelResults of the most recent run (for test.py)

LAG = 4  # steps between an exp tile and its o-chain consumption


def _exp_pattern(n_act=19, n_dve=13):
    """Weighted round-robin ACT/DVE assignment for exp tiles (per 32)."""
    counts = {"A": float(n_act), "D": float(n_dve)}
    total = sum(counts.values())
    acc = dict.fromkeys(counts, 0.0)
    seq = []
    for _ in range(int(total)):
        for k in counts:
            acc[k] += counts[k] / total
        pick = max(acc, key=lambda k: acc[k])
        acc[pick] -= 1.0
        seq.append(pick)
    return seq


def _body(ctx, tc, out_d, xh_d, xt_d, wf_d, wg_d, wh_d, wv_d, id_d):
    nc = tc.nc
    const = ctx.enter_context(tc.tile_pool(name="const", bufs=1))
    big = ctx.enter_context(tc.tile_pool(name="big", bufs=1))
    ep = ctx.enter_context(tc.tile_pool(name="ep", bufs=32))
    obp = ctx.enter_context(tc.tile_pool(name="obp", bufs=6))
    otp = ctx.enter_context(tc.tile_pool(name="otp", bufs=6))
    spp = ctx.enter_context(tc.tile_pool(name="spp", bufs=2))
    ocp = ctx.enter_context(tc.tile_pool(name="ocp", bufs=2))
    rcp = ctx.enter_context(tc.tile_pool(name="rcp", bufs=6))
    fin = ctx.enter_context(tc.tile_pool(name="fin", bufs=16))
    psA = ctx.enter_context(tc.tile_pool(name="psA", bufs=2, space="PSUM"))
    psD = ctx.enter_context(tc.tile_pool(name="psD", bufs=3, space="PSUM"))
    psO = ctx.enter_context(tc.tile_pool(name="psO", bufs=1, space="PSUM"))

    # ---- tiny weights first (instant transfers) on the ACT HWDGE queue ----
    w_sb = {}
    for name, wd in (("f", wf_d), ("g", wg_d), ("h", wh_d)):
        wb = const.tile([P, 2, D], FP16, tag=f"w{name}")
        nc.scalar.dma_start(wb[:], wd.rearrange("c p d -> p c d"))
        w_sb[name] = wb
    wvr = const.tile([P, C], BF16)
    nc.scalar.dma_start(wvr[:], wv_d)
    ident = const.tile([P, P], BF16)
    nc.scalar.dma_start(ident[:], id_d)

    xt = big.tile([P, 2, N], FP16)  # xT: [c (2 chunks of 128), m]
    pieces = [(0, 512), (512, 1024), (1024, 2048), (2048, 3072), (3072, 4096)]
    for a, b in pieces:
        for ch in range(2):
            nc.sync.dma_start(xt[:, ch, a:b], xt_d[ch, :, a:b])
    x_half = big.tile([P, NT, C], FP32)
    qt = big.tile([D, N], FP16)            # q: [d, m]
    kt = big.tile([D, NH], FP16)           # k: [d, n] (own half only)
    v_sb = big.tile([P, MT, D + 1], BF16)  # v: [m, d | 1]
    nc.vector.memset(v_sb[:, :, D:D + 1], 1.0)

    def proj_mm(w, mg, nm):
        pp = psD.tile([D, 512], FP32, tag="pd", name=f"p{nm}{mg}")
        for ch in range(2):
            nc.tensor.matmul(
                pp[:], w[:, ch, :], xt[:, ch, mg * 512:(mg + 1) * 512],
                start=(ch == 0), stop=(ch == 1),
            )
        return pp

    def proj_copy(pp, dst, mg, on_act=False):
        if on_act:
            nc.scalar.copy(dst[:, mg * 512:(mg + 1) * 512], pp[:])
        else:
            nc.vector.tensor_copy(dst[:, mg * 512:(mg + 1) * 512], pp[:])

    def proj(w, dst, mg, nm, on_act=False):
        proj_copy(proj_mm(w, mg, nm), dst, mg, on_act)

    def v_mm(mtg):
        pv = psA.tile([P, 4, D], FP32, tag="pa", name=f"pv{mtg}")
        for j in range(4):
            mt = mtg * 4 + j
            for ch in range(2):
                nc.tensor.matmul(
                    pv[:, j, :], xt[:, ch, mt * P:(mt + 1) * P],
                    w_sb["h"][:, ch, :],
                    start=(ch == 0), stop=(ch == 1),
                )
        return pv

    def v_copy(pv, mtg, on_act=False):
        if on_act:
            nc.scalar.copy(v_sb[:, mtg * 4:(mtg + 1) * 4, 0:D], pv[:])
        else:
            nc.vector.tensor_copy(v_sb[:, mtg * 4:(mtg + 1) * 4, 0:D], pv[:])

    def v_batch(mtg, on_act=False):
        v_copy(v_mm(mtg), mtg, on_act)

    # PE p-state warmup: tiny matmuls on a memset tile (no DMA dependency)
    wsrc = big.tile([P, D], BF16, tag="wsrc")
    nc.vector.memset(wsrc[:], 0.25)
    warm = psA.tile([P, 64], FP32, tag="pa", name="warm")
    for _ in range(32):
        nc.tensor.matmul(
            warm[0:D, 0:D], wsrc[:], wsrc[:, 0:D],
            start=True, stop=True, skip_group_check=True,
        )
    nc.vector.tensor_copy(v_sb[0:D, 0, 0:D], warm[0:D, 0:D])  # keep it live

    # prologue projections (ACT is otherwise idle this early)
    pq0 = proj_mm(w_sb["f"], 0, "q")
    pk0 = proj_mm(w_sb["g"], 0, "k")
    proj_copy(pq0, qt, 0, on_act=True)
    proj_copy(pk0, kt, 0, on_act=False)
    pq1 = proj_mm(w_sb["f"], 1, "q")
    pk1 = proj_mm(w_sb["g"], 1, "k")
    proj_copy(pq1, qt, 1, on_act=True)
    proj_copy(pk1, kt, 1, on_act=False)
    v_batch(0, on_act=True)
    xh_src = xh_d.rearrange("(s p) c -> p s c", p=P)

    pat0 = _exp_pattern(19, 13)   # sg0: DVE busy with staged proj copies
    pat1 = _exp_pattern(19, 13)   # sg1: DVE freer

    def o_mms(sg, j, oB, ets):
        # one accumulation term (query tile j) for all 8 chains of the
        # supergroup; the chains share one PSUM zero-region, so only the
        # very first matmul starts it and the very last stops it (bytes
        # zero lazily on first touch)
        for t in range(8):
            if len(ets) == 1:
                esrc = ets[0][:, t * P:(t + 1) * P]
            else:
                esrc = ets[t // 4][:, (t % 4) * P:(t % 4 + 1) * P]
            nc.tensor.matmul(
                oB[:, t, :], esrc, v_sb[:, j, :],
                start=(j == 0 and t == 0), stop=(j == MT - 1 and t == 7),
            )

    # ---- software-pipelined epilogue stages (issued >=1 step after deps) ----
    import collections
    sched = collections.defaultdict(list)

    def defer(step, fn):
        sched[step].append(fn)

    def emit_finals(sg, first_step, spread):
        """Stage the obars/transpose/final pipeline for supergroup sg.
        Each stage is issued `spread` steps after its producer so every
        instruction's deps are satisfied at issue time (no head-of-line
        blocking in the in-order engine queues)."""
        st = first_step
        ctx2 = {}

        def recs(oB):
            def f():
                rec = rcp.tile([P, 8], FP32, tag="rec", name=f"rec{sg}")
                nc.vector.reciprocal(rec[:], oB[:, :, D])
                ctx2["rec"] = rec
            return f

        def oc_copy(oB, h2):
            # raw (unnormalized) chain outputs -> SBUF bf16, one op per half
            def f():
                oc = obp.tile([P, 4, D], BF16, tag="ob", name=f"oc{sg}_{h2}")
                if h2 == 0:
                    nc.scalar.copy(oc[:], oB[:, h2 * 4:(h2 + 1) * 4, 0:D])
                else:
                    nc.vector.tensor_copy(oc[:], oB[:, h2 * 4:(h2 + 1) * 4, 0:D])
                ctx2[("oc", h2)] = oc
            return f

        def ot_mms(h2):
            # single matmul transposes all 4 bands: lhsT free dims (4, 32)
            # stack onto the 128 output partitions
            def f():
                oTps = psD.tile([P, P], FP32, tag="pd", name=f"otp{sg}_{h2}")
                nc.tensor.matmul(oTps[:], ctx2[("oc", h2)][:], ident[:],
                                 start=True, stop=True)
                ctx2[("otp", h2)] = oTps
            return f

        def ot_copy(h2):
            def f():
                oT = otp.tile([P, P], BF16, tag="ot", name=f"ot{sg}_{h2}")
                if h2 == 0:
                    nc.scalar.copy(oT[:], ctx2[("otp", h2)][:])
                else:
                    nc.vector.tensor_copy(oT[:], ctx2[("otp", h2)][:])
                ctx2[("ot", h2)] = oT
            return f

        def sp_move(h2):
            # band 3 lands at partition 96 which matmul lhsT cannot address;
            # transpose it again separately to a base-0 tile via PE
            def f():
                sp_ps = psD.tile([D, P], FP32, tag="pd", name=f"spp{sg}_{h2}")
                nc.tensor.matmul(sp_ps[:], ctx2[("oc", h2)][:, 3, :], ident[:],
                                 start=True, stop=True)
                sp = spp.tile([D, P], BF16, tag="sp", name=f"sp{sg}_{h2}")
                if h2 == 0:
                    nc.scalar.copy(sp[:], sp_ps[:])
                else:
                    nc.vector.tensor_copy(sp[:], sp_ps[:])
                ctx2[("sp", h2)] = sp
            return f

        def f_mm(h2, bd):
            def f():
                nt = sg * 8 + h2 * 4 + bd
                # tail finals can also use the (then idle) ACT-lane slots
                fpool, ftag = (psA, "pa") if (sg == 1 and bd % 2 == 1) else (psD, "pd")
                fps = fpool.tile([P, C], FP32, tag=ftag, name=f"F{nt}")
                if bd < 3:
                    nc.tensor.matmul(fps[:],
                                     ctx2[("ot", h2)][bd * D:(bd + 1) * D, :],
                                     wvr[bd * D:(bd + 1) * D, :],
                                     start=True, stop=True)
                else:
                    nc.tensor.matmul(fps[:], ctx2[("sp", h2)][:], wvr[0:D, :],
                                     start=True, stop=True)
                ctx2[("f", h2, bd)] = fps
            return f

        def f_scale_add(h2, bd):
            def f():
                nt = sg * 8 + h2 * 4 + bd
                t = h2 * 4 + bd
                fps = ctx2[("f", h2, bd)]
                rec = ctx2["rec"]
                osb = fin.tile([P, C], FP32, tag="osb", name=f"osb{nt}")
                if bd % 2 == 0:
                    nc.scalar.activation(osb[:], fps[:],
                                         mybir.ActivationFunctionType.Copy,
                                         scale=rec[:, t:t + 1])
                    nc.gpsimd.tensor_add(osb[:], osb[:], x_half[:, nt, :])
                else:
                    nc.vector.tensor_scalar(osb[:], fps[:], rec[:, t:t + 1],
                                            None, Aop.mult)
                    nc.vector.tensor_add(osb[:], osb[:], x_half[:, nt, :])
                ctx2[("osb", h2, bd)] = osb
            return f

        def f_dma(h2, bd):
            def f():
                nt = sg * 8 + h2 * 4 + bd
                osb = ctx2[("osb", h2, bd)]
                if sg == 1 and bd == 3:
                    dq = nc.gpsimd   # pool-queue: idle engine, parallel path
                elif sg == 1 and h2 == 1:
                    dq = nc.scalar
                else:
                    dq = nc.sync
                dq.dma_start(out_d[nt * P:(nt + 1) * P, :], osb[:])
            return f

        def f_out(h2, bd):
            def f():
                f_scale_add(h2, bd)()
                f_dma(h2, bd)()
            return f

        oB = oBs[sg]
        if spread == 0:
            # dense tail: interleave both halves level-by-level so their
            # stage chains run concurrently on different engines
            defer(st, recs(oB))
            for h2 in range(2):
                defer(st, oc_copy(oB, h2))
            for h2 in range(2):
                defer(st, ot_mms(h2))
            for h2 in range(2):
                defer(st, ot_copy(h2))
            for h2 in range(2):
                defer(st, sp_move(h2))
            for bd in range(4):
                for h2 in range(2):
                    defer(st, f_mm(h2, bd))
            for bd in range(4):
                for h2 in range(2):
                    defer(st, f_scale_add(h2, bd))
            for bd in range(4):
                for h2 in range(2):
                    defer(st, f_dma(h2, bd))
        else:
            defer(st, recs(oB))
            for h2 in range(2):
                b = st + (1 + h2 * 5) * spread
                defer(b, oc_copy(oB, h2))
                defer(b + spread, ot_mms(h2))
                defer(b + 2 * spread, ot_copy(h2))
                defer(b + 2 * spread, sp_move(h2))
                for bd in range(4):
                    defer(b + (3 + bd) * spread, f_mm(h2, bd))
                    defer(b + (4 + bd) * spread, f_out(h2, bd))

    # ---- main loop: flat over 64 beta/exp tiles, o-chains lag LAG steps ----
    oBs = [None, None]
    etiles = {}

    def o_step(s):
        sgp, j = divmod(s, MT)
        if j == 0:
            oBs[sgp] = psO.tile([P, 8, D + 1], FP32, tag="o", name=f"oB{sgp}")
        o_mms(sgp, j, oBs[sgp], etiles.pop(s))

    # x_half loads are only needed by the finals; put them on the sync queue
    # BEHIND the critical xt pieces (queues dispatch strictly in order)
    for piece in range(4):
        nc.sync.dma_start(
            x_half[:, piece * 4:(piece + 1) * 4, :],
            xh_src[:, piece * 4:(piece + 1) * 4, :],
        )

    # staged projection work: (kind, idx, mm_step); copy issues next step.
    # deadlines: q mg by step 4*mg-1; v mtg by step 4*mtg+LAG-1; k by 31
    stages = [
        ("v", 1, 4), ("q", 2, 5),
        ("v", 2, 7), ("q", 3, 9),
        ("k", 2, 10), ("v", 3, 11), ("q", 4, 13),
        ("k", 3, 14), ("v", 4, 15), ("q", 5, 17),
        ("v", 5, 19), ("q", 6, 21),
        ("v", 6, 23), ("q", 7, 25),
        ("v", 7, 27),
    ]
    for kind, idx, st in stages:
        def mk(kind, idx):
            def mm():
                if kind == "q":
                    etiles[("p", kind, idx)] = proj_mm(w_sb["f"], idx, "q")
                elif kind == "k":
                    etiles[("p", kind, idx)] = proj_mm(w_sb["g"], idx, "k")
                else:
                    etiles[("p", kind, idx)] = v_mm(idx)

            def cp():
                pp = etiles.pop(("p", kind, idx))
                if kind == "q":
                    proj_copy(pp, qt, idx)
                elif kind == "k":
                    proj_copy(pp, kt, idx)
                else:
                    v_copy(pp, idx, on_act=True)
            return mm, cp
        mm, cp = mk(kind, idx)
        defer(st, mm)
        defer(st + 1, cp)

    # finals for sg0 run spread through sg1; sg1's run densely at the end
    emit_finals_done = [False, False]

    NSTEPS = MT * SG
    for s in range(NSTEPS):
        sg, mt = divmod(s, MT)
        lane = (pat0 if sg == 0 else pat1)[s % 32]
        if lane == "A":
            et = ep.tile([P, SGW], BF16, tag="e", name=f"e{s}")
            etiles[s] = (et,)
            pb = psA.tile([P, SGW], FP32, tag="pa", name=f"pb{s}")
            for hf in range(2):
                nc.tensor.matmul(
                    pb[:, hf * 512:(hf + 1) * 512],
                    qt[:, mt * P:(mt + 1) * P],
                    kt[:, sg * SGW + hf * 512:sg * SGW + (hf + 1) * 512],
                    start=True, stop=True,
                )
            nc.scalar.activation(et[:], pb[:],
                                 mybir.ActivationFunctionType.Exp)
        else:
            ets = []
            for hf in range(2):
                eh = ep.tile([P, 512], BF16, tag="e", name=f"e{s}_{hf}")
                pb = psD.tile([P, 512], FP32, tag="pd", name=f"pb{s}_{hf}")
                nc.tensor.matmul(
                    pb[:],
                    qt[:, mt * P:(mt + 1) * P],
                    kt[:, sg * SGW + hf * 512:sg * SGW + (hf + 1) * 512],
                    start=True, stop=True,
                )
                nc.vector.tensor_scalar(
                    eh[:].bitcast(I16), pb[:],
                    EXP_S1, EXP_S2, Aop.mult, Aop.add)
                ets.append(eh)
            etiles[s] = tuple(ets)
        if s >= LAG:
            o_step(s - LAG)
            if s - LAG == MT - 1 and not emit_finals_done[0]:
                emit_finals(0, s + 1, 1)
                emit_finals_done[0] = True
        for fn in sched.pop(s, []):
            fn()
    # tail: trailing o-steps, then sg1 finals densely
    for s in range(NSTEPS - LAG, NSTEPS):
        o_step(s)
    emit_finals(1, NSTEPS, 0)
    for st in sorted(sched):
        for fn in sched.pop(st):
            fn()


def build_program():
    nc = bacc.Bacc(
        "TRN2",
        target_bir_lowering=False,
        debug=False,
        enable_asserts=False,
        num_devices=NCORES,
    )
    xh_d = nc.dram_tensor("xh", [NH, C], FP32, kind="ExternalInput").ap()
    xt_d = nc.dram_tensor("xt", [2, P, N], FP16, kind="ExternalInput").ap()
    wf_d = nc.dram_tensor("Wf16", [2, P, D], FP16, kind="ExternalInput").ap()
    wg_d = nc.dram_tensor("Wg16", [2, P, D], FP16, kind="ExternalInput").ap()
    wh_d = nc.dram_tensor("Wh16", [2, P, D], FP16, kind="ExternalInput").ap()
    wv_d = nc.dram_tensor("WvG", [P, C], BF16, kind="ExternalInput").ap()
    id_d = nc.dram_tensor("Ident", [P, P], BF16, kind="ExternalInput").ap()
    out_d = nc.dram_tensor("out", [NH, C], FP32, kind="ExternalOutput").ap()

    with tile.TileContext(nc) as tc:
        with ExitStack() as ctx:
            _body(ctx, tc, out_d, xh_d, xt_d, wf_d, wg_d, wh_d, wv_d, id_d)
    nc.compile()
    return nc


_CACHE = {}


def _get_program():
    if "nc" not in _CACHE:
        _CACHE["nc"] = build_program()
    return _CACHE["nc"]


def make_in_maps(inputs):
    x = np.ascontiguousarray(np.asarray(inputs["x"], np.float32)).reshape(B, N, C)
    gam = np.float32(np.asarray(inputs["gamma"], np.float32).reshape(()))
    w16 = {}
    for nm in ("Wf", "Wg", "Wh"):
        w = np.asarray(inputs[nm], np.float32).astype(np.float16)  # [256, 32]
        w16[nm] = np.ascontiguousarray(w.reshape(2, P, D))
    wv1 = (gam * np.asarray(inputs["Wv"], np.float32)).astype(ml_dtypes.bfloat16)
    wv = np.ascontiguousarray(np.tile(wv1, (4, 1)))
    ident = np.ascontiguousarray(np.eye(P, dtype=ml_dtypes.bfloat16))

    in_maps = []
    for c in range(NCORES):
        b, h = divmod(c, 2)
        if h == 0:
            xb = x[b]
        else:
            xb = np.concatenate([x[b, NH:], x[b, :NH]], axis=0)
        xt = np.ascontiguousarray(xb.T.astype(np.float16).reshape(2, P, N))
        in_maps.append(
            {
                "xh": np.ascontiguousarray(xb[:NH]),
                "xt": xt,
                "Wf16": w16["Wf"],
                "Wg16": w16["Wg"],
                "Wh16": w16["Wh"],
                "WvG": wv,
                "Ident": ident,
            }
        )
    return in_maps


def kernel(**inputs):
    global LAST_RESULTS
    nc = _get_program()
    in_maps = make_in_maps(inputs)
    res = run_bass_kernel_spmd(nc, in_maps, core_ids=list(range(NCORES)))
    LAST_RESULTS = res
    out = np.empty((B, N, C), np.float32)
    for c in range(NCORES):
        b, h = divmod(c, 2)
        out[b, h * NH:(h + 1) * NH] = res.results[c]["out"]
    return out.reshape(B, H, W, C)


# revision 56
# speedup vs baseline: 1.4340x; 1.0128x over previous
"""NonLocalBlock (self-attention over 64x64 image, C=256, D=32) on 8 trn2 cores.

Sharding: data-parallel over B=4 batches x 2-way split of the attention
rows (the `n` axis of beta[n, m]) => 8 cores, each computing a [2048, 256]
slice of the output. Each core receives its batch image pre-transposed
(and fp16-cast) by the host, rolled so its own 2048 rows come first,
plus its own half in natural layout for the residual. The host also
pre-casts the 1x1-conv weights and folds gamma into Wv.

Device math (per core, n = its 2048 key rows, m = all 4096 queries):
  logits[m, n] = q_m . k_n               PE, fp16, [128m x 512n] matmuls
  E[m, n] = exp(logits)                  ACT exact exp + DVE exp2 bit-trick
                                         (int16(l*128*log2e + 16250.875)
                                          bitcast to bf16, trunc-fitted)
  o[n, :] = sum_m E[m, n] v_aug[m, :]    PE, E stationary (33 cols/matmul),
                                         terms issued a few steps behind
                                         each exp tile; 8 chains share one
                                         PSUM zero-region (single start/stop)
  obar = o[:, 0:32] / o[:, 32]           DVE reciprocal + ACT/DVE scale, bf16
  oT = transpose(obar)                   PE (identity-moving), 32-row bands
  out[n, :] = oT.T @ (gamma Wv) + x      PE matmul + {ACT copy + Pool add |
                                         DVE fused add}, DMA out

Engine layout: ACT exp tiles are [128,1024] double-buffered in PSUM
(2x2 banks), DVE trick tiles are [128,512] double-buffered (2x1 banks),
o-chain accumulator 1 bank, final tiles 1 bank.
"""

from contextlib import ExitStack

import ml_dtypes
import numpy as np

import concourse.bass as bass
import concourse.tile as tile
from concourse import bacc, mybir
from concourse.bass_utils import run_bass_kernel_spmd

B, H, W, C = 4, 64, 64, 256
N = H * W            # 4096 pixels per image
D = 32               # reduced channel dim
NH = N // 2          # key rows owned by each core
P = 128
MT = N // P          # 32 query (m) tiles
NT = NH // P         # 16 n-tiles of 128 per core
SG = 2               # supergroups of 1024 n-columns
SGW = NH // SG       # 1024
FP32 = mybir.dt.float32
BF16 = mybir.dt.bfloat16
FP16 = mybir.dt.float16
I16 = mybir.dt.int16
NCORES = 8

# exp(l) ~= bf16-bitcast(int16(l * 128*log2(e) + 16250.875)); the int16
# convert truncates, constant fitted for that (max rel err 3.3%)
EXP_S1 = float(np.float32(128 * 1.4426950408889634))
EXP_S2 = 16250.875
Aop = mybir.AluOpType

LAST_RESULTS = None  # BassKernelResults of the most recent run (for test.py)

LAG = 6  # steps between an exp tile and its o-chain consumption


def _exp_pattern(n_act=19, n_dve=13):
    """Weighted round-robin ACT/DVE assignment for exp tiles (per 32)."""
    counts = {"A": float(n_act), "D": float(n_dve)}
    total = sum(counts.values())
    acc = dict.fromkeys(counts, 0.0)
    seq = []
    for _ in range(int(total)):
        for k in counts:
            acc[k] += counts[k] / total
        pick = max(acc, key=lambda k: acc[k])
        acc[pick] -= 1.0
        seq.append(pick)
    return seq


def _body(ctx, tc, out_d, xh_d, xt_d, wf_d, wg_d, wh_d, wv_d, id_d):
    nc = tc.nc
    const = ctx.enter_context(tc.tile_pool(name="const", bufs=1))
    big = ctx.enter_context(tc.tile_pool(name="big", bufs=1))
    ep = ctx.enter_context(tc.tile_pool(name="ep", bufs=32))
    obp = ctx.enter_context(tc.tile_pool(name="obp", bufs=6))
    otp = ctx.enter_context(tc.tile_pool(name="otp", bufs=6))
    spp = ctx.enter_context(tc.tile_pool(name="spp", bufs=2))
    ocp = ctx.enter_context(tc.tile_pool(name="ocp", bufs=2))
    rcp = ctx.enter_context(tc.tile_pool(name="rcp", bufs=6))
    fin = ctx.enter_context(tc.tile_pool(name="fin", bufs=16))
    psA = ctx.enter_context(tc.tile_pool(name="psA", bufs=2, space="PSUM"))
    psD = ctx.enter_context(tc.tile_pool(name="psD", bufs=3, space="PSUM"))
    psO = ctx.enter_context(tc.tile_pool(name="psO", bufs=1, space="PSUM"))

    # ---- tiny weights first (instant transfers) on the ACT HWDGE queue ----
    w_sb = {}
    for name, wd in (("f", wf_d), ("g", wg_d), ("h", wh_d)):
        wb = const.tile([P, 2, D], FP16, tag=f"w{name}")
        nc.scalar.dma_start(wb[:], wd.rearrange("c p d -> p c d"))
        w_sb[name] = wb
    wvr = const.tile([P, C], BF16)
    nc.scalar.dma_start(wvr[:], wv_d)
    ident = const.tile([P, P], BF16)
    nc.scalar.dma_start(ident[:], id_d)

    xt = big.tile([P, 2, N], FP16)  # xT: [c (2 chunks of 128), m]
    pieces = [(0, 512), (512, 1024), (1024, 2048), (2048, 3072), (3072, 4096)]
    for a, b in pieces:
        for ch in range(2):
            nc.sync.dma_start(xt[:, ch, a:b], xt_d[ch, :, a:b])
    x_half = big.tile([P, NT, C], FP32)
    qt = big.tile([D, N], FP16)            # q: [d, m]
    kt = big.tile([D, NH], FP16)           # k: [d, n] (own half only)
    v_sb = big.tile([P, MT, D + 1], BF16)  # v: [m, d | 1]
    nc.vector.memset(v_sb[:, :, D:D + 1], 1.0)

    def proj_mm(w, mg, nm):
        pp = psD.tile([D, 512], FP32, tag="pd", name=f"p{nm}{mg}")
        for ch in range(2):
            nc.tensor.matmul(
                pp[:], w[:, ch, :], xt[:, ch, mg * 512:(mg + 1) * 512],
                start=(ch == 0), stop=(ch == 1),
            )
        return pp

    def proj_copy(pp, dst, mg, on_act=False):
        if on_act:
            nc.scalar.copy(dst[:, mg * 512:(mg + 1) * 512], pp[:])
        else:
            nc.vector.tensor_copy(dst[:, mg * 512:(mg + 1) * 512], pp[:])

    def proj(w, dst, mg, nm, on_act=False):
        proj_copy(proj_mm(w, mg, nm), dst, mg, on_act)

    def v_mm(mtg):
        pv = psA.tile([P, 4, D], FP32, tag="pa", name=f"pv{mtg}")
        for j in range(4):
            mt = mtg * 4 + j
            for ch in range(2):
                nc.tensor.matmul(
                    pv[:, j, :], xt[:, ch, mt * P:(mt + 1) * P],
                    w_sb["h"][:, ch, :],
                    start=(ch == 0), stop=(ch == 1),
                )
        return pv

    def v_copy(pv, mtg, on_act=False):
        if on_act:
            nc.scalar.copy(v_sb[:, mtg * 4:(mtg + 1) * 4, 0:D], pv[:])
        else:
            nc.vector.tensor_copy(v_sb[:, mtg * 4:(mtg + 1) * 4, 0:D], pv[:])

    def v_batch(mtg, on_act=False):
        v_copy(v_mm(mtg), mtg, on_act)

    # PE p-state warmup: tiny matmuls on a memset tile (no DMA dependency)
    wsrc = big.tile([P, D], BF16, tag="wsrc")
    nc.vector.memset(wsrc[:], 0.25)
    warm = psA.tile([P, 64], FP32, tag="pa", name="warm")
    for _ in range(32):
        nc.tensor.matmul(
            warm[0:D, 0:D], wsrc[:], wsrc[:, 0:D],
            start=True, stop=True, skip_group_check=True,
        )
    nc.vector.tensor_copy(v_sb[0:D, 0, 0:D], warm[0:D, 0:D])  # keep it live

    # prologue projections (ACT is otherwise idle this early)
    pq0 = proj_mm(w_sb["f"], 0, "q")
    pk0 = proj_mm(w_sb["g"], 0, "k")
    proj_copy(pq0, qt, 0, on_act=True)
    proj_copy(pk0, kt, 0, on_act=False)
    pq1 = proj_mm(w_sb["f"], 1, "q")
    pk1 = proj_mm(w_sb["g"], 1, "k")
    proj_copy(pq1, qt, 1, on_act=True)
    proj_copy(pk1, kt, 1, on_act=False)
    v_batch(0, on_act=True)
    xh_src = xh_d.rearrange("(s p) c -> p s c", p=P)

    pat0 = _exp_pattern(19, 13)   # sg0: DVE busy with staged proj copies
    pat1 = _exp_pattern(19, 13)   # sg1: DVE freer

    def o_mms(sg, j, oB, ets):
        # one accumulation term (query tile j) for all 8 chains of the
        # supergroup; the chains share one PSUM zero-region, so only the
        # very first matmul starts it and the very last stops it (bytes
        # zero lazily on first touch)
        for t in range(8):
            if len(ets) == 1:
                esrc = ets[0][:, t * P:(t + 1) * P]
            else:
                esrc = ets[t // 4][:, (t % 4) * P:(t % 4 + 1) * P]
            nc.tensor.matmul(
                oB[:, t, :], esrc, v_sb[:, j, :],
                start=(j == 0 and t == 0), stop=(j == MT - 1 and t == 7),
            )

    # ---- software-pipelined epilogue stages (issued >=1 step after deps) ----
    import collections
    sched = collections.defaultdict(list)

    def defer(step, fn):
        sched[step].append(fn)

    def emit_finals(sg, first_step, spread):
        """Stage the obars/transpose/final pipeline for supergroup sg.
        Each stage is issued `spread` steps after its producer so every
        instruction's deps are satisfied at issue time (no head-of-line
        blocking in the in-order engine queues)."""
        st = first_step
        ctx2 = {}

        def recs(oB):
            def f():
                rec = rcp.tile([P, 8], FP32, tag="rec", name=f"rec{sg}")
                nc.vector.reciprocal(rec[:], oB[:, :, D])
                ctx2["rec"] = rec
            return f

        def oc_copy(oB, h2):
            # raw (unnormalized) chain outputs -> SBUF bf16, one op per half
            def f():
                oc = obp.tile([P, 4, D], BF16, tag="ob", name=f"oc{sg}_{h2}")
                if h2 == 0:
                    nc.scalar.copy(oc[:], oB[:, h2 * 4:(h2 + 1) * 4, 0:D])
                else:
                    nc.vector.tensor_copy(oc[:], oB[:, h2 * 4:(h2 + 1) * 4, 0:D])
                ctx2[("oc", h2)] = oc
            return f

        def ot_mms(h2):
            # single matmul transposes all 4 bands: lhsT free dims (4, 32)
            # stack onto the 128 output partitions
            def f():
                oTps = psD.tile([P, P], FP32, tag="pd", name=f"otp{sg}_{h2}")
                nc.tensor.matmul(oTps[:], ctx2[("oc", h2)][:], ident[:],
                                 start=True, stop=True)
                ctx2[("otp", h2)] = oTps
            return f

        def ot_copy(h2):
            def f():
                oT = otp.tile([P, P], BF16, tag="ot", name=f"ot{sg}_{h2}")
                if h2 == 0:
                    nc.scalar.copy(oT[:], ctx2[("otp", h2)][:])
                else:
                    nc.vector.tensor_copy(oT[:], ctx2[("otp", h2)][:])
                ctx2[("ot", h2)] = oT
            return f

        def sp_move(h2):
            # band 3 lands at partition 96 which matmul lhsT cannot address;
            # transpose it again separately to a base-0 tile via PE
            def f():
                sp_ps = psD.tile([D, P], FP32, tag="pd", name=f"spp{sg}_{h2}")
                nc.tensor.matmul(sp_ps[:], ctx2[("oc", h2)][:, 3, :], ident[:],
                                 start=True, stop=True)
                sp = spp.tile([D, P], BF16, tag="sp", name=f"sp{sg}_{h2}")
                if h2 == 0:
                    nc.scalar.copy(sp[:], sp_ps[:])
                else:
                    nc.vector.tensor_copy(sp[:], sp_ps[:])
                ctx2[("sp", h2)] = sp
            return f

        def f_mm(h2, bd):
            def f():
                nt = sg * 8 + h2 * 4 + bd
                # tail finals can also use the (then idle) ACT-lane slots
                fpool, ftag = (psA, "pa") if (sg == 1 and bd % 2 == 1) else (psD, "pd")
                fps = fpool.tile([P, C], FP32, tag=ftag, name=f"F{nt}")
                if bd < 3:
                    nc.tensor.matmul(fps[:],
                                     ctx2[("ot", h2)][bd * D:(bd + 1) * D, :],
                                     wvr[bd * D:(bd + 1) * D, :],
                                     start=True, stop=True)
                else:
                    nc.tensor.matmul(fps[:], ctx2[("sp", h2)][:], wvr[0:D, :],
                                     start=True, stop=True)
                ctx2[("f", h2, bd)] = fps
            return f

        def f_scale_add(h2, bd):
            def f():
                nt = sg * 8 + h2 * 4 + bd
                t = h2 * 4 + bd
                fps = ctx2[("f", h2, bd)]
                rec = ctx2["rec"]
                osb = fin.tile([P, C], FP32, tag="osb", name=f"osb{nt}")
                if bd % 2 == 0:
                    nc.scalar.activation(osb[:], fps[:],
                                         mybir.ActivationFunctionType.Copy,
                                         scale=rec[:, t:t + 1])
                    nc.gpsimd.tensor_add(osb[:], osb[:], x_half[:, nt, :])
                else:
                    nc.vector.tensor_scalar(osb[:], fps[:], rec[:, t:t + 1],
                                            None, Aop.mult)
                    nc.vector.tensor_add(osb[:], osb[:], x_half[:, nt, :])
                ctx2[("osb", h2, bd)] = osb
            return f

        def f_dma(h2, bd):
            def f():
                nt = sg * 8 + h2 * 4 + bd
                osb = ctx2[("osb", h2, bd)]
                if sg == 1 and bd == 3:
                    dq = nc.gpsimd   # pool-queue: idle engine, parallel path
                elif sg == 1 and h2 == 1:
                    dq = nc.scalar
                else:
                    dq = nc.sync
                dq.dma_start(out_d[nt * P:(nt + 1) * P, :], osb[:])
            return f

        def f_out(h2, bd):
            def f():
                f_scale_add(h2, bd)()
                f_dma(h2, bd)()
            return f

        oB = oBs[sg]
        if spread == 0:
            # dense tail: interleave both halves level-by-level so their
            # stage chains run concurrently on different engines
            defer(st, recs(oB))
            for h2 in range(2):
                defer(st, oc_copy(oB, h2))
            for h2 in range(2):
                defer(st, ot_mms(h2))
            for h2 in range(2):
                defer(st, ot_copy(h2))
            for h2 in range(2):
                defer(st, sp_move(h2))
            for bd in range(4):
                for h2 in range(2):
                    defer(st, f_mm(h2, bd))
            for bd in range(4):
                for h2 in range(2):
                    defer(st, f_scale_add(h2, bd))
            for bd in range(4):
                for h2 in range(2):
                    defer(st, f_dma(h2, bd))
        else:
            defer(st, recs(oB))
            for h2 in range(2):
                b = st + (1 + h2 * 5) * spread
                defer(b, oc_copy(oB, h2))
                defer(b + spread, ot_mms(h2))
                defer(b + 2 * spread, ot_copy(h2))
                defer(b + 2 * spread, sp_move(h2))
                for bd in range(4):
                    defer(b + (3 + bd) * spread, f_mm(h2, bd))
                    defer(b + (4 + bd) * spread, f_out(h2, bd))

    # ---- main loop: flat over 64 beta/exp tiles, o-chains lag LAG steps ----
    oBs = [None, None]
    etiles = {}

    def o_step(s):
        sgp, j = divmod(s, MT)
        if j == 0:
            oBs[sgp] = psO.tile([P, 8, D + 1], FP32, tag="o", name=f"oB{sgp}")
        o_mms(sgp, j, oBs[sgp], etiles.pop(s))

    # x_half loads are only needed by the finals; put them on the sync queue
    # BEHIND the critical xt pieces (queues dispatch strictly in order)
    for piece in range(4):
        nc.sync.dma_start(
            x_half[:, piece * 4:(piece + 1) * 4, :],
            xh_src[:, piece * 4:(piece + 1) * 4, :],
        )

    # staged projection work: (kind, idx, mm_step); copy issues next step.
    # deadlines: q mg by step 4*mg-1; v mtg by step 4*mtg+LAG-1; k by 31
    stages = [
        ("v", 1, 4), ("q", 2, 5),
        ("v", 2, 7), ("q", 3, 9),
        ("k", 2, 10), ("v", 3, 11), ("q", 4, 13),
        ("k", 3, 14), ("v", 4, 15), ("q", 5, 17),
        ("v", 5, 19), ("q", 6, 21),
        ("v", 6, 23), ("q", 7, 25),
        ("v", 7, 27),
    ]
    for kind, idx, st in stages:
        def mk(kind, idx):
            def mm():
                if kind == "q":
                    etiles[("p", kind, idx)] = proj_mm(w_sb["f"], idx, "q")
                elif kind == "k":
                    etiles[("p", kind, idx)] = proj_mm(w_sb["g"], idx, "k")
                else:
                    etiles[("p", kind, idx)] = v_mm(idx)

            def cp():
                pp = etiles.pop(("p", kind, idx))
                if kind == "q":
                    proj_copy(pp, qt, idx)
                elif kind == "k":
                    proj_copy(pp, kt, idx)
                else:
                    v_copy(pp, idx, on_act=True)
            return mm, cp
        mm, cp = mk(kind, idx)
        defer(st, mm)
        defer(st + 1, cp)

    # finals for sg0 run spread through sg1; sg1's run densely at the end
    emit_finals_done = [False, False]

    NSTEPS = MT * SG
    for s in range(NSTEPS):
        sg, mt = divmod(s, MT)
        lane = (pat0 if sg == 0 else pat1)[s % 32]
        if lane == "A":
            et = ep.tile([P, SGW], BF16, tag="e", name=f"e{s}")
            etiles[s] = (et,)
            pb = psA.tile([P, SGW], FP32, tag="pa", name=f"pb{s}")
            for hf in range(2):
                nc.tensor.matmul(
                    pb[:, hf * 512:(hf + 1) * 512],
                    qt[:, mt * P:(mt + 1) * P],
                    kt[:, sg * SGW + hf * 512:sg * SGW + (hf + 1) * 512],
                    start=True, stop=True,
                )
            nc.scalar.activation(et[:], pb[:],
                                 mybir.ActivationFunctionType.Exp)
        else:
            ets = []
            for hf in range(2):
                eh = ep.tile([P, 512], BF16, tag="e", name=f"e{s}_{hf}")
                pb = psD.tile([P, 512], FP32, tag="pd", name=f"pb{s}_{hf}")
                nc.tensor.matmul(
                    pb[:],
                    qt[:, mt * P:(mt + 1) * P],
                    kt[:, sg * SGW + hf * 512:sg * SGW + (hf + 1) * 512],
                    start=True, stop=True,
                )
                nc.vector.tensor_scalar(
                    eh[:].bitcast(I16), pb[:],
                    EXP_S1, EXP_S2, Aop.mult, Aop.add)
                ets.append(eh)
            etiles[s] = tuple(ets)
        if s >= LAG:
            o_step(s - LAG)
            if s - LAG == MT - 1 and not emit_finals_done[0]:
                emit_finals(0, s + 1, 2)
                emit_finals_done[0] = True
        for fn in sched.pop(s, []):
            fn()
    # tail: trailing o-steps, then sg1 finals densely
    for s in range(NSTEPS - LAG, NSTEPS):
        o_step(s)
    emit_finals(1, NSTEPS, 0)
    for st in sorted(sched):
        for fn in sched.pop(st):
            fn()


def build_program():
    nc = bacc.Bacc(
        "TRN2",
        target_bir_lowering=False,
        debug=False,
        enable_asserts=False,
        num_devices=NCORES,
    )
    xh_d = nc.dram_tensor("xh", [NH, C], FP32, kind="ExternalInput").ap()
    xt_d = nc.dram_tensor("xt", [2, P, N], FP16, kind="ExternalInput").ap()
    wf_d = nc.dram_tensor("Wf16", [2, P, D], FP16, kind="ExternalInput").ap()
    wg_d = nc.dram_tensor("Wg16", [2, P, D], FP16, kind="ExternalInput").ap()
    wh_d = nc.dram_tensor("Wh16", [2, P, D], FP16, kind="ExternalInput").ap()
    wv_d = nc.dram_tensor("WvG", [P, C], BF16, kind="ExternalInput").ap()
    id_d = nc.dram_tensor("Ident", [P, P], BF16, kind="ExternalInput").ap()
    out_d = nc.dram_tensor("out", [NH, C], FP32, kind="ExternalOutput").ap()

    with tile.TileContext(nc) as tc:
        with ExitStack() as ctx:
            _body(ctx, tc, out_d, xh_d, xt_d, wf_d, wg_d, wh_d, wv_d, id_d)
    nc.compile()
    return nc


_CACHE = {}


def _get_program():
    if "nc" not in _CACHE:
        _CACHE["nc"] = build_program()
    return _CACHE["nc"]


def make_in_maps(inputs):
    x = np.ascontiguousarray(np.asarray(inputs["x"], np.float32)).reshape(B, N, C)
    gam = np.float32(np.asarray(inputs["gamma"], np.float32).reshape(()))
    w16 = {}
    for nm in ("Wf", "Wg", "Wh"):
        w = np.asarray(inputs[nm], np.float32).astype(np.float16)  # [256, 32]
        w16[nm] = np.ascontiguousarray(w.reshape(2, P, D))
    wv1 = (gam * np.asarray(inputs["Wv"], np.float32)).astype(ml_dtypes.bfloat16)
    wv = np.ascontiguousarray(np.tile(wv1, (4, 1)))
    ident = np.ascontiguousarray(np.eye(P, dtype=ml_dtypes.bfloat16))

    in_maps = []
    for c in range(NCORES):
        b, h = divmod(c, 2)
        if h == 0:
            xb = x[b]
        else:
            xb = np.concatenate([x[b, NH:], x[b, :NH]], axis=0)
        xt = np.ascontiguousarray(xb.T.astype(np.float16).reshape(2, P, N))
        in_maps.append(
            {
                "xh": np.ascontiguousarray(xb[:NH]),
                "xt": xt,
                "Wf16": w16["Wf"],
                "Wg16": w16["Wg"],
                "Wh16": w16["Wh"],
                "WvG": wv,
                "Ident": ident,
            }
        )
    return in_maps


def kernel(**inputs):
    global LAST_RESULTS
    nc = _get_program()
    in_maps = make_in_maps(inputs)
    res = run_bass_kernel_spmd(nc, in_maps, core_ids=list(range(NCORES)))
    LAST_RESULTS = res
    out = np.empty((B, N, C), np.float32)
    for c in range(NCORES):
        b, h = divmod(c, 2)
        out[b, h * NH:(h + 1) * NH] = res.results[c]["out"]
    return out.reshape(B, H, W, C)
